# revision 1
# baseline (speedup 1.0000x reference)
"""Causal self-attention Trainium2 kernel.

Problem: y = CausalSelfAttention(x) with B=4, T=2048, C=1024, H=16 heads,
head_dim D=64, qkv split order (k, q, v), softmax scale C**-0.5.

Sharding (8 cores): core = 2*b + g  -> batch b in 0..3, head-group g in 0..1
(8 heads per group).  Each core computes, for its batch and its 8 heads:
  qkv partial matmuls, causal attention, and the partial output projection
  y_partial = att_out @ W_proj[rows of this head group].
The host sums the two partial projections per batch (row-parallel tensor
parallelism reduced on host during unsharding).

Device layout notes (per core):
  xT    [128, 8, 2048]  x^T (C on partitions), loaded via DMA transpose (bf16)
  kqT   [128, 8, 2048]  (x @ W_kq)^T : blocks 0-3 = k-channels, 4-7 = q-channels
                         head h: 64*(h%2) partition offset, block h//2 (+4 for q)
  v_aug [128, 16, 520]  v in natural layout, 65 cols/head = [v(64) | ones(1)]
  S^T   [k partitions, q free] -> exp on ACT (scale 1/32, fp32 PSUM -> bf16)
        full 128k-blocks computed in per-head pairs (2-bank PSUM tile per
        pair); diagonal-band blocks col-sliced to valid columns and paired
        across the two concurrently-processed heads; the triangular mask is
        zeroed via gpsimd affine_select on the leading 128 columns.
  AV:   out^T[65, q] = [V|1]^T @ P^T accumulated over k tiles; row 64 = softmax
        denominator.  reciprocal (DVE) -> shift to partition 0 (DMA) ->
        partition_broadcast (gpsimd) -> multiply (DVE) -> place into att
        (SBUF->SBUF DMA, handles the odd-head partition offset).
  proj: y^T[1024, 2048] = W_proj_g(lhsT) @ att^T, streamed to HBM in fp32.

Scheduling: qkv for head-pair hp+1 is emitted interleaved with the attention
chunks of head-pair hp (separate PSUM tag) so the PE fills ACT-bound exp
windows with qkv matmuls.
"""

import numpy as np
import ml_dtypes

B, T, C, H = 4, 2048, 1024, 16
D = C // H          # 64
HPC = H // 2        # 8 heads per core
CG = C // 2         # 512 channels per head group
P = 128

_compiled = {}


def _build(t=T):
    import concourse.bacc as bacc
    import concourse.tile as tile
    import concourse.mybir as mybir

    f32 = mybir.dt.float32
    bf16 = mybir.dt.bfloat16
    Exp = mybir.ActivationFunctionType.Exp

    KT = C // P            # 8 contraction tiles over C
    MB = (2 * CG) // P     # 8 kq channel blocks (0-3 k, 4-7 q)
    TT = t // P            # token tiles of 128
    QC = t // 512          # q chunks of 512
    VB = CG // P           # 4 v/att channel blocks
    SCALE = float(C) ** -0.5

    nc = bacc.Bacc("TRN2", target_bir_lowering=False, debug=False,
                   num_devices=8)

    x_d = nc.dram_tensor("x", [t, C], bf16, kind="ExternalInput")
    wkq_d = nc.dram_tensor("wkq", [C, 2 * CG], bf16, kind="ExternalInput")
    wv_d = nc.dram_tensor("wv", [C, CG], bf16, kind="ExternalInput")
    wp_d = nc.dram_tensor("wp", [CG, C], bf16, kind="ExternalInput")
    y_d = nc.dram_tensor("y", [C, t], f32, kind="ExternalOutput")

    with tile.TileContext(nc) as tc:
        with (
            tc.tile_pool(name="persist", bufs=1) as persist,
            tc.tile_pool(name="psA", bufs=2, space="PSUM") as psA,
            tc.tile_pool(name="avP", bufs=1, space="PSUM") as avP,
            tc.tile_pool(name="ptP", bufs=16) as ptP,
            tc.tile_pool(name="ptdP", bufs=8) as ptdP,
            tc.tile_pool(name="rcP", bufs=2) as rcP,
            tc.tile_pool(name="rbP", bufs=2) as rbP,
            tc.tile_pool(name="atP", bufs=3) as atP,
            tc.tile_pool(name="yP", bufs=3) as yP,
        ):
            xT = persist.tile([P, KT, t], bf16)
            wkq_sb = persist.tile([P, KT, 2 * CG], bf16)
            wv_sb = persist.tile([P, KT, CG], bf16)
            wp_sb = persist.tile([P, VB, C], bf16)
            kqT = persist.tile([P, MB, t], bf16)
            v_aug = persist.tile([P, TT, HPC * (D + 1)], bf16)
            att = persist.tile([P, VB, t], bf16)

            dma_engs = [nc.sync, nc.sync]

            # PE warm-up: dependency-free matmuls run during the input-DMA
            # window so the HAM clock gate is at 8/8 when real work starts.
            wu_a = persist.tile([P, P], bf16)
            wu_b = persist.tile([P, 512], bf16)
            nc.vector.memset(wu_a, 0.0)
            nc.vector.memset(wu_b, 0.0)
            for _ in range(44):
                wps = psA.tile([P, 512], f32, name="wups", tag="qp", bufs=2)
                nc.tensor.matmul(wps, lhsT=wu_a, rhs=wu_b,
                                 start=True, stop=True,
                                 skip_group_check=True)

            # ---- loads: split across both HWDGE queues ----
            for ct in range(KT):
                dma_engs[ct % 2].dma_start(
                    xT[:, ct, :], x_d[:, ct * P:(ct + 1) * P], transpose=True)
            wkq_r = wkq_d.ap().rearrange("(kt p) m -> p kt m", p=P)
            wv_r = wv_d.ap().rearrange("(kt p) m -> p kt m", p=P)
            wp_r = wp_d.ap().rearrange("(kt p) m -> p kt m", p=P)
            for kt in range(KT):
                nc.sync.dma_start(wkq_sb[:, kt, :], wkq_r[:, kt, :])
                nc.sync.dma_start(wv_sb[:, kt, :], wv_r[:, kt, :])
            for kt in range(VB):
                nc.sync.dma_start(wp_sb[:, kt, :], wp_r[:, kt, :])
            nc.vector.memset(v_aug, 1.0)

            def chunk_pairs(n):
                return [list(range(i, min(i + 2, n))) for i in range(0, n, 2)]

            # one qkv "unit" = one PSUM accumulation group; "st"-tag units
            # use a 2-chunk (2-bank) tile, "qp"-tag units a 1-chunk tile
            def emit_kq_unit(mb, grp, tag):
                nu = 2 if tag == "st" else 1
                grp = grp if tag == "st" else grp[:1]
                ps = psA.tile([P, nu, 512], f32, name="ps", tag=tag,
                              bufs=2)
                for kt in range(KT):
                    for u, c in enumerate(grp):
                        nc.tensor.matmul(
                            ps[:, u, :],
                            lhsT=wkq_sb[:, kt, mb * P:(mb + 1) * P],
                            rhs=xT[:, kt, c * 512:(c + 1) * 512],
                            start=(kt == 0), stop=(kt == KT - 1),
                            skip_group_check=True)
                nc.vector.tensor_copy(
                    kqT[:, mb, grp[0] * 512:(grp[-1] + 1) * 512],
                    ps[:, 0:len(grp), :].rearrange("p u n -> p (u n)"))

            def emit_v_unit(grp, tag):
                nu = 2 if tag == "st" else 1
                grp = grp if tag == "st" else grp[:1]
                ps = psA.tile([P, nu, CG], f32, name="psv", tag=tag,
                              bufs=2)
                for kt in range(KT):
                    for u, tt in enumerate(grp):
                        nc.tensor.matmul(
                            ps[:, u, :],
                            lhsT=xT[:, kt, tt * P:(tt + 1) * P],
                            rhs=wv_sb[:, kt, :],
                            start=(kt == 0), stop=(kt == KT - 1),
                            skip_group_check=True)
                nc.vector.tensor_copy(
                    v_aug[:, grp[0]:grp[-1] + 1, :].rearrange(
                        "p u (h e) -> p u h e", e=D + 1)[:, :, :, 0:D],
                    ps[:, 0:len(grp), :].rearrange(
                        "p u (h d) -> p u h d", d=D))

            def emit_attn_chunk(hp, c):
                nfull = 4 * c
                avp = [avP.tile([D + 1, 512], f32, name=f"avp{hi}",
                                tag="avp", bufs=2)
                       for hi in range(2)]
                work = [[], []]
                for j in range(nfull):
                    # both heads' S^T for k-tile j in one 2-bank tile: the
                    # two matmuls are PE-adjacent with different row groups
                    # (rows 0:64 vs 64:128) so the systolic array overlaps
                    # them; one exp covers both heads
                    st = psA.tile([P, 2, 512], f32, name="st", tag="st")
                    for hi in range(2):
                        lo = D * hi
                        nc.tensor.matmul(
                            st[:, hi, :],
                            lhsT=kqT[lo:lo + D, hp, j * P:(j + 1) * P],
                            rhs=kqT[lo:lo + D, 4 + hp,
                                    c * 512:(c + 1) * 512],
                            start=True, stop=True,
                            skip_group_check=True)
                    pt = ptP.tile([P, 2, 512], bf16, name="pt", tag="pt")
                    nc.scalar.activation(pt, st, Exp, scale=SCALE)
                    for hi in range(2):
                        work[hi].append((pt[:, hi, :], j, 0))
                for dj in range(4):
                    j = nfull + dj
                    off = P * dj
                    w = 512 - off
                    st = psA.tile([P, 2, 512], f32, name="std", tag="st")
                    for hi in range(2):
                        lo = D * hi
                        nc.tensor.matmul(
                            st[:, hi, 0:w],
                            lhsT=kqT[lo:lo + D, hp, j * P:(j + 1) * P],
                            rhs=kqT[lo:lo + D, 4 + hp,
                                    c * 512 + off:(c + 1) * 512],
                            start=True, stop=True,
                            skip_group_check=True)
                    pt = ptdP.tile([P, 2, 512], bf16, name="ptd", tag="ptd")
                    nc.scalar.activation(pt[:, :, 0:w], st[:, :, 0:w],
                                         Exp, scale=SCALE)
                    nc.gpsimd.affine_select(
                        pt[:, :, 0:P], pt[:, :, 0:P],
                        pattern=[[0, 2], [1, P]],
                        compare_op=mybir.AluOpType.is_ge,
                        fill=0.0, base=0, channel_multiplier=-1)
                    for hi in range(2):
                        work[hi].append((pt[:, hi, 0:w], j, off))
                for hi in range(2):
                    h = 2 * hp + hi
                    n = len(work[hi])
                    for idx, (pap, j, off) in enumerate(work[hi]):
                        out_ap = avp[hi][:, off:512] if off else avp[hi]
                        nc.tensor.matmul(
                            out_ap,
                            lhsT=v_aug[:, j, h * (D + 1):(h + 1) * (D + 1)],
                            rhs=pap,
                            start=(idx == 0), stop=(idx == n - 1),
                            skip_group_check=True)
                    rc = rcP.tile([D + 1, 512], f32)
                    nc.vector.reciprocal(rc[D:D + 1, :], avp[hi][D:D + 1, :])
                    # partition_broadcast only reads physical partition 0;
                    # DMA-shift the reciprocal row there first.
                    rc0 = rcP.tile([1, 512], f32, name="rc0", tag="rc0")
                    nc.sync.dma_start(rc0, rc[D:D + 1, :])
                    rb = rbP.tile([D, 512], f32)
                    nc.gpsimd.partition_broadcast(rb, rc0[0:1, :], channels=D)
                    at = atP.tile([D, 512], bf16)
                    nc.vector.tensor_mul(at, avp[hi][0:D, :], rb)
                    nc.sync.dma_start(
                        att[D * hi:D * (hi + 1), hp, c * 512:(c + 1) * 512],
                        at)

            # ---- startup: just enough for attn(0, 0..1), alternate tags ----
            cps = chunk_pairs(QC)
            vps = chunk_pairs(TT)
            startup = [("kq", 0, cps[0]), ("kq", 4, cps[0])]
            startup += [("v", None, g) for g in vps[0:2]]
            for i, (kind, mb, grp) in enumerate(startup):
                if kind == "kq":
                    emit_kq_unit(mb, grp, "st")
                else:
                    emit_v_unit(grp, "st")

            # Remaining qkv/v units (single-chunk, 1-bank "qp" tiles),
            # emitted as PE filler between attention chunks.  Tile discovers
            # dependencies from TRACE order, so a producer MUST be emitted
            # before its first consumer chunk; each fill carries the global
            # chunk index it is first needed by.
            def cdiv(a, b):
                return -(-a // b)

            fills = []
            for tt in range(4, TT):
                # attn(0, c) AV reads v tiles tt <= 4c+3
                fills.append((max(0, cdiv(tt - 3, 4)), ("v", None, [tt])))
            for hp in range(4):
                for ck in range(QC):
                    if hp == 0 and ck in (0, 1):
                        continue
                    # k-side: attn(hp, c) reads j-tiles <= 4c+3 of block hp
                    fills.append((4 * hp + ck, ("kq", hp, [ck])))
                    # q-side: attn(hp, c) reads q chunk c of block 4+hp
                    fills.append((4 * hp + ck, ("kq", 4 + hp, [ck])))
            fills.sort(key=lambda f: f[0])

            # ---- attention with interleaved filler units ----
            nchunks = 4 * QC
            emitted = 0

            def emit_fills(upto):
                nonlocal emitted
                while emitted < min(upto, len(fills)):
                    _, (kind, mb, grp) = fills[emitted]
                    if kind == "kq":
                        emit_kq_unit(mb, grp, "qp")
                    else:
                        emit_v_unit(grp, "qp")
                    emitted += 1

            for hp in range(4):
                for c in range(QC):
                    ci = hp * QC + c
                    # everything this chunk reads must already be emitted
                    while emitted < len(fills) and fills[emitted][0] <= ci:
                        emit_fills(emitted + 1)
                    emit_attn_chunk(hp, c)
                    emit_fills(((ci + 4) * len(fills)) // nchunks)
            emit_fills(len(fills))

            # ---- projection: y^T = W_proj_g(lhsT) @ att^T ----
            # gi-major so the first-half chunks (ready before the final
            # attention chunks finish) are emitted first
            for grp in chunk_pairs(QC):
                for mb in range(C // P):
                    ps = psA.tile([P, 2, 512], f32, name="psp", tag="st",
                                  bufs=2)
                    for kt in range(VB):
                        for u, c in enumerate(grp):
                            nc.tensor.matmul(
                                ps[:, u, :],
                                lhsT=wp_sb[:, kt, mb * P:(mb + 1) * P],
                                rhs=att[:, kt, c * 512:(c + 1) * 512],
                                start=(kt == 0), stop=(kt == VB - 1),
                                skip_group_check=True)
                    yt = yP.tile([P, 2, 512], f32)
                    nc.vector.tensor_copy(yt[:, 0:len(grp), :],
                                          ps[:, 0:len(grp), :])
                    nc.sync.dma_start(
                        y_d[mb * P:(mb + 1) * P,
                            grp[0] * 512:(grp[-1] + 1) * 512],
                        yt[:, 0:len(grp), :].rearrange("p u n -> p (u n)"))

    nc.compile()
    return nc


def _get_compiled(t=T):
    if t not in _compiled:
        _compiled[t] = _build(t)
    return _compiled[t]


def make_in_maps(x, W_qkv, W_proj):
    bf = ml_dtypes.bfloat16
    x = np.asarray(x, dtype=np.float32)
    W_qkv = np.asarray(W_qkv, dtype=np.float32)
    W_proj = np.asarray(W_proj, dtype=np.float32)
    in_maps = []
    for core in range(8):
        b, g = core // 2, core % 2
        in_maps.append({
            "x": np.ascontiguousarray(x[b]).astype(bf),
            "wkq": np.concatenate(
                [W_qkv[:, g * CG:(g + 1) * CG],
                 W_qkv[:, C + g * CG:C + (g + 1) * CG]], axis=1).astype(bf),
            "wv": np.ascontiguousarray(
                W_qkv[:, 2 * C + g * CG:2 * C + (g + 1) * CG]).astype(bf),
            "wp": np.ascontiguousarray(
                W_proj[g * CG:(g + 1) * CG, :]).astype(bf),
        })
    return in_maps


def _run_axon_nodonate(nc, in_maps, n_cores=8):
    """Execute via PJRT/shard_map WITHOUT output-buffer donation.

    bass2jax.run_bass_via_pjrt donates the zero output operands; under the
    axon transport that donation intermittently corrupts multi-core results.
    This kernel writes every element of its output, so donation is not
    needed for correctness -- pass non-donated zero operands instead.
    """
    import jax
    from jax.sharding import Mesh, PartitionSpec
    from jax.experimental.shard_map import shard_map
    import concourse.mybir as mybir
    from concourse.bass2jax import _bass_exec_p, install_neuronx_cc_hook

    install_neuronx_cc_hook()
    in_names, out_names, out_avals = [], [], []
    for alloc in nc.m.functions[0].allocations:
        if not isinstance(alloc, mybir.MemoryLocationSet):
            continue
        name = alloc.memorylocations[0].name
        if alloc.kind == "ExternalInput":
            in_names.append(name)
        elif alloc.kind == "ExternalOutput":
            out_names.append(name)
            out_avals.append(jax.core.ShapedArray(
                tuple(alloc.tensor_shape), mybir.dt.np(alloc.dtype)))
    n_params = len(in_names)
    all_names = in_names + out_names
    pid_name = nc.partition_id_tensor.name if nc.partition_id_tensor else None

    def _body(*args):
        return tuple(_bass_exec_p.bind(
            *args,
            out_avals=tuple(out_avals),
            in_names=tuple(all_names),
            out_names=tuple(out_names),
            lowering_input_output_aliases=(),
            sim_require_finite=True,
            sim_require_nnan=True,
            nc=nc,
        ))

    devices = jax.devices()[:n_cores]
    mesh = Mesh(np.asarray(devices), ("core",))
    fn = jax.jit(
        shard_map(_body, mesh=mesh,
                  in_specs=(PartitionSpec("core"),) * (n_params + len(out_names)),
                  out_specs=(PartitionSpec("core"),) * len(out_names),
                  check_rep=False),
        keep_unused=True)
    concat_in = [
        np.concatenate([
            np.asarray(in_maps[c].get(
                nm, np.array([[c]], dtype=np.uint32) if nm == pid_name
                else None))
            for c in range(n_cores)], 0)
        for nm in in_names
    ]
    concat_zeros = [
        np.zeros((n_cores * a.shape[0], *a.shape[1:]), a.dtype)
        for a in out_avals
    ]
    out = fn(*concat_in, *concat_zeros)
    return [
        {nm: np.asarray(out[i]).reshape(n_cores, *out_avals[i].shape)[c]
         for i, nm in enumerate(out_names)}
        for c in range(n_cores)
    ]


def kernel(x, W_qkv, W_proj, _trace=False):
    from concourse._compat import axon_active

    nc = _get_compiled()
    in_maps = make_in_maps(x, W_qkv, W_proj)
    if axon_active():
        results = _run_axon_nodonate(nc, in_maps)
    else:
        import concourse.bass_utils as bass_utils
        res = bass_utils.run_bass_kernel_spmd(
            nc, in_maps, core_ids=list(range(8)), trace=_trace)
        if _trace:
            kernel.last_results = res
        results = res.results
    y = np.zeros((B, T, C), np.float32)
    for core in range(8):
        y[core // 2] += results[core]["y"].T
    return y



# revision 10
# speedup vs baseline: 1.2119x; 1.2119x over previous
"""Causal self-attention Trainium2 kernel (fp8 DoubleRow + natural-AV).

Problem: y = CausalSelfAttention(x) with B=4, T=2048, C=1024, H=16 heads,
head_dim D=64, qkv split order (k, q, v), softmax scale C**-0.5.

Sharding (8 cores): core = 2*b + g  -> batch b in 0..3, head-group g in 0..1
(8 heads per group).  Each core computes, for its batch and its 8 heads:
qkv partial matmuls, causal attention, and the partial output projection
y_partial = att_out @ W_proj[rows of this head group].  The host sums the two
partial projections per batch.

Key speed tricks (tuned against the concourse instruction cost model):
  * k/q QKV matmuls run in fp8(e4m3) with perf_mode=DoubleRow: each
    instruction contracts TWO 128-deep k-tiles at 0.5 cycles/row.
    Weights are pre-scaled by WS=32 on the host so fp8 keeps precision;
    the scale is folded into the softmax exp scale (1/WS^2).
  * S = q^T k runs in fp8 DoubleRow too: D=64 is split into two 32-halves
    stored at different free offsets on quarter partition ranges
    (head h lives on partitions 32*(h%4)..+31).  The host permutes the
    W_qkv columns so the QKV matmul output lands directly in this layout
    (PSUM->SBUF copies stay partition-preserving).
  * AV runs in the natural [q, d] orientation: lhsT = P^T block (exp
    output), rhs = [V_h | ones] so N=65 streamed rows per k-tile instead
    of 512, and causal sparsity is exploited per 128-token q-tile.
    The ones column gives the softmax denominator; a per-partition
    reciprocal + broadcast multiply normalizes.  Four q-tile accumulation
    chains share one PSUM bank (only the first matmul in the bank sets
    start=True; the bank-wide pending-zero covers the other chains).
  * att comes out token-major; PE transpose (identity matmul) flips it to
    channel-major for the bf16 projection.
  * v and proj stay bf16: fp8 there would inject ~2-3% output error.
  * The attention stream is ACT(exp)-bound.  Emission interleaves, at
    S-block granularity: S blocks of chunk X+1, AV chains of chunk X, and
    qkv/v/transpose/proj filler units paced so that emitted PE rows track
    emitted ACT row-equivalents (keeps the PE p-state ramp hot and the
    ACT queue never empty).
"""

import numpy as np
import ml_dtypes

B, T, C, H = 4, 2048, 1024, 16
D = C // H          # 64
HPC = H // 2        # 8 heads per core
CG = C // 2         # 512 channels per head group
P = 128
KT = C // P         # 8 contraction tiles over C
TT = T // P         # 16 token tiles
QC = T // 512       # 4 q chunks of 512
VB = CG // P        # 4 att/channel blocks
WS = 32.0           # fp8 weight pre-scale for k/q

_compiled = {}


def _build(t=T):
    import concourse.bacc as bacc
    import concourse.tile as tile
    import concourse.mybir as mybir

    f32 = mybir.dt.float32
    bf16 = mybir.dt.bfloat16
    f8 = mybir.dt.float8e4
    Exp = mybir.ActivationFunctionType.Exp
    DR = mybir.MatmulPerfMode.DoubleRow

    tt_n = t // P
    qc_n = t // 512
    SCALE = float(C) ** -0.5 / (WS * WS)

    nc = bacc.Bacc("TRN2", target_bir_lowering=False, debug=False,
                   num_devices=8)

    xT_d = nc.dram_tensor("xT", [C, t], bf16, kind="ExternalInput")
    x8_d = nc.dram_tensor("x8T", [C, t], f8, kind="ExternalInput")
    wkq_d = nc.dram_tensor("wkq8", [C, 8, P], f8, kind="ExternalInput")
    wv_d = nc.dram_tensor("wv", [C, CG], bf16, kind="ExternalInput")
    wp_d = nc.dram_tensor("wp", [CG, C], bf16, kind="ExternalInput")
    id_d = nc.dram_tensor("ident", [P, P], bf16, kind="ExternalInput")
    y_d = nc.dram_tensor("y", [C, t], f32, kind="ExternalOutput")

    with tile.TileContext(nc) as tc:
        with (
            tc.tile_pool(name="persist", bufs=1) as persist,
            tc.tile_pool(name="psA", bufs=2, space="PSUM") as psA,
            tc.tile_pool(name="ptP", bufs=16) as ptP,
            tc.tile_pool(name="ptdP", bufs=8) as ptdP,
            tc.tile_pool(name="rcP", bufs=4) as rcP,
            tc.tile_pool(name="ytP", bufs=2) as ytP,
        ):
            xT = persist.tile([P, KT, t], bf16)
            x8 = persist.tile([P, KT, t], f8)
            wkq_sb = persist.tile([P, KT, 8, P], f8)
            wv_sb = persist.tile([P, KT, CG], bf16)
            wp_sb = persist.tile([P, VB, C], bf16)
            id_sb = persist.tile([P, P], bf16)
            # kq8[32Q+r, kq, s, e, tok] = (x @ Wkq*WS)[tok, 64*(Q+4s)+32e+r]
            kq8 = persist.tile([P, 2, 2, 2, t], f8)
            # engine APs can only start at partition 0/32/64, so the Q=3
            # quarter (partitions 96..127) is DMA-duplicated to base 0 here
            kq8b = persist.tile([32, 2, 2, 2, t], f8)
            v_aug = persist.tile([P, tt_n, HPC, D + 1], bf16)
            att_nat = persist.tile([P, tt_n, HPC, D], bf16)
            attT = persist.tile([P, VB, t], bf16)

            # PE warm-up: dependency-free matmuls run during the input-DMA
            # window so the p-state ramp is hot when real work starts.
            wu_a = persist.tile([P, P], bf16)
            wu_b = persist.tile([P, 512], bf16)
            nc.vector.memset(wu_a, 0.0)
            nc.vector.memset(wu_b, 0.0)
            for _ in range(40):
                wps = psA.tile([P, 512], f32, name="wups", tag="qp", bufs=2)
                nc.tensor.matmul(wps, lhsT=wu_a, rhs=wu_b,
                                 start=True, stop=True,
                                 skip_group_check=True)

            # ---- input loads ----
            wkq_r = wkq_d.ap().rearrange("(kt p) b m -> p kt b m", p=P)
            nc.sync.dma_start(wkq_sb, wkq_r)
            x8_r = x8_d.ap().rearrange("(kt p) n -> p kt n", p=P)
            for i in range(4):
                nc.sync.dma_start(x8[:, 2 * i:2 * i + 2, :],
                                  x8_r[:, 2 * i:2 * i + 2, :])
            wv_r = wv_d.ap().rearrange("(kt p) m -> p kt m", p=P)
            nc.sync.dma_start(wv_sb, wv_r)
            xT_r = xT_d.ap().rearrange("(kt p) n -> p kt n", p=P)
            for i in range(2):
                nc.sync.dma_start(xT[:, 4 * i:4 * i + 4, :],
                                  xT_r[:, 4 * i:4 * i + 4, :])
            wp_r = wp_d.ap().rearrange("(kt p) m -> p kt m", p=P)
            nc.sync.dma_start(wp_sb, wp_r)
            nc.sync.dma_start(id_sb, id_d.ap())
            nc.vector.memset(v_aug, 1.0)

            # ---- pacing state ----
            st8 = {"pe": 0, "act": 0}

            def pe(rows):
                st8["pe"] += rows

            def act(rows):
                st8["act"] += rows

            # ---- unit emitters ----
            def emit_kq_unit(kq, s, e, c):
                """One 128-out-channel block of k or q for one 512-tok
                chunk, fp8 DoubleRow over kt pairs."""
                blk = 4 * kq + 2 * s + e
                ps = psA.tile([P, 512], f32, name="ps", tag="qp", bufs=2)
                for kt in range(4):
                    nc.tensor.matmul(
                        ps,
                        lhsT=wkq_sb[:, 2 * kt:2 * kt + 2, blk, :],
                        rhs=x8[:, 2 * kt:2 * kt + 2, c * 512:(c + 1) * 512],
                        start=(kt == 0), stop=(kt == 3),
                        perf_mode=DR, skip_group_check=True)
                nc.vector.tensor_copy(
                    kq8[:, kq, s, e, c * 512:(c + 1) * 512], ps)
                nc.sync.dma_start(
                    kq8b[:, kq, s, e, c * 512:(c + 1) * 512],
                    kq8[96:128, kq, s, e, c * 512:(c + 1) * 512])
                pe(1024)

            def emit_v_unit(tt):
                """v for one 128-token tile, bf16, natural layout."""
                ps = psA.tile([P, CG], f32, name="psv", tag="qp", bufs=2)
                for kt in range(KT):
                    nc.tensor.matmul(
                        ps,
                        lhsT=xT[:, kt, tt * P:(tt + 1) * P],
                        rhs=wv_sb[:, kt, :],
                        start=(kt == 0), stop=(kt == KT - 1),
                        skip_group_check=True)
                nc.vector.tensor_copy(
                    v_aug[:, tt, :, 0:D],
                    ps.rearrange("p (h d) -> p h d", d=D))
                pe(4096)

            def emit_transpose(tt, cb):
                """att_nat[:, tt, heads 2cb..2cb+1] -> attT channel-major."""
                pst = psA.tile([P, P], bf16, name="pst", tag="qp", bufs=2)
                nc.tensor.transpose(
                    pst, att_nat[:, tt, 2 * cb:2 * cb + 2, :], id_sb)
                nc.vector.tensor_copy(attT[:, cb, tt * P:(tt + 1) * P], pst)
                pe(128)

            def emit_proj_unit(mb, c):
                ps = psA.tile([P, 512], f32, name="psp", tag="qp", bufs=2)
                for kt in range(VB):
                    nc.tensor.matmul(
                        ps,
                        lhsT=wp_sb[:, kt, mb * P:(mb + 1) * P],
                        rhs=attT[:, kt, c * 512:(c + 1) * 512],
                        start=(kt == 0), stop=(kt == VB - 1),
                        skip_group_check=True)
                yt = ytP.tile([P, 512], f32, name="yt", tag="yt", bufs=2)
                nc.vector.tensor_copy(yt, ps)
                nc.sync.dma_start(
                    y_d[mb * P:(mb + 1) * P, c * 512:(c + 1) * 512], yt)
                pe(2048)

            # ---- filler queue ----
            fills = []  # (fn, key) pairs; key = (kind, idx)
            for c in range(1, qc_n):
                for kq in (0, 1):
                    for s in (0, 1):
                        for e in (0, 1):
                            fills.append(
                                ((lambda kq=kq, s=s, e=e, c=c:
                                  emit_kq_unit(kq, s, e, c)), ("kq", c)))
                for tt in range(4 * c, 4 * c + 4):
                    fills.append(((lambda tt=tt: emit_v_unit(tt)),
                                  ("v", tt)))

            def emit_fills():
                """Top up fillers until emitted PE rows track ACT rows."""
                while fills and st8["pe"] < st8["act"]:
                    fn, key = fills.pop(0)
                    fn()

            def force_fills(kind, idx):
                i = 0
                while i < len(fills):
                    fn, key = fills[i]
                    if key[0] == kind and key[1] <= idx:
                        fills.pop(i)
                        fn()
                    else:
                        i += 1

            # ---- startup: kq chunk 0 for all heads, v tiles 0..3 ----
            for s in (0, 1):
                for e in (0, 1):
                    emit_kq_unit(0, s, e, 0)
                    emit_kq_unit(1, s, e, 0)
            for tt in range(4):
                emit_v_unit(tt)

            # ---- attention building blocks ----
            def s_block_emitters(hp, c):
                """Per-block closures for S^T + exp of head pair hp,
                chunk c; returns (emitters, pts) where pts is filled in
                as blocks run."""
                s_h = hp // 2
                q0, q1 = (2 * hp) % 4, (2 * hp + 1) % 4
                nfull = 4 * c
                pts = []
                ems = []

                def kq_op(kq, Q, c0, c1):
                    if Q == 3:
                        return kq8b[:, kq, s_h, :, c0:c1]
                    return kq8[32 * Q:32 * Q + 32, kq, s_h, :, c0:c1]

                def full_block(j):
                    def em():
                        st = psA.tile([P, 2, 512], f32, name="st", tag="st",
                                      bufs=2)
                        for hi, Q in ((0, q0), (1, q1)):
                            nc.tensor.matmul(
                                st[:, hi, :],
                                lhsT=kq_op(0, Q, j * P, (j + 1) * P),
                                rhs=kq_op(1, Q, c * 512, (c + 1) * 512),
                                start=True, stop=True,
                                perf_mode=DR, skip_group_check=True)
                        pt = ptP.tile([P, 2, 512], bf16, name="pt",
                                      tag="pt")
                        nc.scalar.activation(pt, st, Exp, scale=SCALE)
                        pts.append((pt, j, 0))
                        pe(512)
                        act(2 * 1024 + 444)
                    return em

                def diag_block(dj):
                    def em():
                        j = nfull + dj
                        off = P * dj
                        w = 512 - off
                        st = psA.tile([P, 2, 512], f32, name="std",
                                      tag="st")
                        for hi, Q in ((0, q0), (1, q1)):
                            nc.tensor.matmul(
                                st[:, hi, 0:w],
                                lhsT=kq_op(0, Q, j * P, (j + 1) * P),
                                rhs=kq_op(1, Q, c * 512 + off,
                                          (c + 1) * 512),
                                start=True, stop=True,
                                perf_mode=DR, skip_group_check=True)
                        pt = ptdP.tile([P, 2, 512], bf16, name="ptd",
                                       tag="ptd")
                        nc.scalar.activation(pt[:, :, 0:w], st[:, :, 0:w],
                                             Exp, scale=SCALE)
                        nc.gpsimd.affine_select(
                            pt[:, :, 0:P], pt[:, :, 0:P],
                            pattern=[[0, 2], [1, P]],
                            compare_op=mybir.AluOpType.is_ge,
                            fill=0.0, base=0, channel_multiplier=-1)
                        pts.append((pt, j, off))
                        pe(w)
                        act(4 * w + 444)
                    return em

                for j in range(nfull):
                    ems.append(full_block(j))
                for dj in range(4):
                    ems.append(diag_block(dj))
                return ems, pts

            def av_emitters(hp, c, pts):
                """Per-head closures: 4 accumulation chains + drain."""
                ems = []

                def head(hi):
                    def em():
                        h = 2 * hp + hi
                        avp = psA.tile([P, 4, P], f32, name=f"avp{hi}",
                                       tag="avp", bufs=2)
                        rows = 0
                        for u in range(4):
                            tq = 4 * c + u
                            chain = [pj for pj in pts if pj[1] <= tq]
                            n = len(chain)
                            for idx, (pt, j, off) in enumerate(chain):
                                lo = u * P - off
                                nc.tensor.matmul(
                                    avp[:, u, 0:D + 1],
                                    lhsT=pt[:, hi, lo:lo + P],
                                    rhs=v_aug[:, j, h, :],
                                    start=(u == 0 and idx == 0),
                                    stop=(idx == n - 1),
                                    skip_group_check=True)
                                rows += D + 1
                        rc = rcP.tile([P, 4, 1], f32, name="rc", tag="rc",
                                      bufs=4)
                        nc.vector.reciprocal(rc, avp[:, :, D:D + 1])
                        nc.vector.tensor_mul(
                            att_nat[:, 4 * c:4 * c + 4, h, :],
                            avp[:, :, 0:D],
                            rc.broadcast_to([P, 4, D]))
                        pe(rows)
                    return em

                return [head(0), head(1)]

            # ---- main interleaved emission ----
            chunk_seq = [(hp, c) for c in range(qc_n) for hp in range(4)]
            pending = None   # (hp, c, av emitter list)
            for hp, c in chunk_seq:
                force_fills("kq", c)
                s_ems, pts = s_block_emitters(hp, c)
                avq = []
                if pending is not None:
                    php, pc, avq = pending
                    force_fills("v", 4 * pc + 3)
                for bi, em in enumerate(s_ems):
                    em()
                    if bi >= 2 and avq:
                        avq.pop(0)()
                    emit_fills()
                while avq:
                    avq.pop(0)()
                    emit_fills()
                if pending is not None and php == 3:
                    # att tokens of chunk pc complete for all 16 heads
                    for tt in range(4 * pc, 4 * pc + 4):
                        for cb in range(VB):
                            fills.append(
                                ((lambda tt=tt, cb=cb:
                                  emit_transpose(tt, cb)), ("tp", pc)))
                    for mb in range(KT):
                        fills.append(
                            ((lambda mb=mb, pc=pc:
                              emit_proj_unit(mb, pc)), ("pj", pc)))
                pending = (hp, c, av_emitters(hp, c, pts))

            php, pc, avq = pending
            force_fills("v", 4 * pc + 3)
            for em in avq:
                em()
            for tt in range(4 * pc, 4 * pc + 4):
                for cb in range(VB):
                    emit_transpose(tt, cb)
            for mb in range(KT):
                emit_proj_unit(mb, pc)
            while fills:
                fn, key = fills.pop(0)
                fn()

    nc.compile()
    return nc


def _get_compiled(t=T):
    if t not in _compiled:
        _compiled[t] = _build(t)
    return _compiled[t]


def make_in_maps(x, W_qkv, W_proj):
    bf = ml_dtypes.bfloat16
    f8 = ml_dtypes.float8_e4m3
    x = np.asarray(x, dtype=np.float32)
    W_qkv = np.asarray(W_qkv, dtype=np.float32)
    W_proj = np.asarray(W_proj, dtype=np.float32)
    ident = np.eye(P, dtype=np.float32).astype(bf)
    in_maps = []
    for core in range(8):
        b, g = core // 2, core % 2
        xT = np.ascontiguousarray(x[b].T)           # [C, T]
        Wk = W_qkv[:, g * CG:(g + 1) * CG]
        Wq = W_qkv[:, C + g * CG:C + (g + 1) * CG]
        Wv = W_qkv[:, 2 * C + g * CG:2 * C + (g + 1) * CG]
        # permute k/q columns into the quarter layout:
        # block (kq, s, e) partition 32Q+r <- column 64*(Q+4s)+32e+r
        wkq8 = np.empty((C, 8, P), dtype=np.float32)
        for kqi, W in ((0, Wk), (1, Wq)):
            Wr = (W * WS).reshape(C, 2, 4, 2, 32)   # [C, s, Q, e, r]
            for s in range(2):
                for e in range(2):
                    blk = 4 * kqi + 2 * s + e
                    wkq8[:, blk, :] = Wr[:, s, :, e, :].reshape(C, P)
        in_maps.append({
            "xT": xT.astype(bf),
            "x8T": xT.astype(f8),
            "wkq8": wkq8.astype(f8),
            "wv": np.ascontiguousarray(Wv).astype(bf),
            "wp": np.ascontiguousarray(
                W_proj[g * CG:(g + 1) * CG, :]).astype(bf),
            "ident": ident,
        })
    return in_maps


def _run_axon_nodonate(nc, in_maps, n_cores=8):
    """Execute via PJRT/shard_map WITHOUT output-buffer donation.

    bass2jax.run_bass_via_pjrt donates the zero output operands; under the
    axon transport that donation intermittently corrupts multi-core results.
    This kernel writes every element of its output, so donation is not
    needed for correctness -- pass non-donated zero operands instead.
    """
    import jax
    from jax.sharding import Mesh, PartitionSpec
    from jax.experimental.shard_map import shard_map
    import concourse.mybir as mybir
    from concourse.bass2jax import _bass_exec_p, install_neuronx_cc_hook

    install_neuronx_cc_hook()
    in_names, out_names, out_avals = [], [], []
    for alloc in nc.m.functions[0].allocations:
        if not isinstance(alloc, mybir.MemoryLocationSet):
            continue
        name = alloc.memorylocations[0].name
        if alloc.kind == "ExternalInput":
            in_names.append(name)
        elif alloc.kind == "ExternalOutput":
            out_names.append(name)
            out_avals.append(jax.core.ShapedArray(
                tuple(alloc.tensor_shape), mybir.dt.np(alloc.dtype)))
    n_params = len(in_names)
    all_names = in_names + out_names
    pid_name = nc.partition_id_tensor.name if nc.partition_id_tensor else None

    def _body(*args):
        return tuple(_bass_exec_p.bind(
            *args,
            out_avals=tuple(out_avals),
            in_names=tuple(all_names),
            out_names=tuple(out_names),
            lowering_input_output_aliases=(),
            sim_require_finite=True,
            sim_require_nnan=True,
            nc=nc,
        ))

    devices = jax.devices()[:n_cores]
    mesh = Mesh(np.asarray(devices), ("core",))
    fn = jax.jit(
        shard_map(_body, mesh=mesh,
                  in_specs=(PartitionSpec("core"),) * (n_params + len(out_names)),
                  out_specs=(PartitionSpec("core"),) * len(out_names),
                  check_rep=False),
        keep_unused=True)
    concat_in = [
        np.concatenate([
            np.asarray(in_maps[c].get(
                nm, np.array([[c]], dtype=np.uint32) if nm == pid_name
                else None))
            for c in range(n_cores)], 0)
        for nm in in_names
    ]
    concat_zeros = [
        np.zeros((n_cores * a.shape[0], *a.shape[1:]), a.dtype)
        for a in out_avals
    ]
    out = fn(*concat_in, *concat_zeros)
    return [
        {nm: np.asarray(out[i]).reshape(n_cores, *out_avals[i].shape)[c]
         for i, nm in enumerate(out_names)}
        for c in range(n_cores)
    ]


def kernel(x, W_qkv, W_proj, _trace=False):
    from concourse._compat import axon_active

    nc = _get_compiled()
    in_maps = make_in_maps(x, W_qkv, W_proj)
    if axon_active():
        results = _run_axon_nodonate(nc, in_maps)
    else:
        import concourse.bass_utils as bass_utils
        res = bass_utils.run_bass_kernel_spmd(
            nc, in_maps, core_ids=list(range(8)), trace=_trace)
        if _trace:
            kernel.last_results = res
        results = res.results
    y = np.zeros((B, T, C), np.float32)
    for core in range(8):
        y[core // 2] += results[core]["y"].T
    return y


# revision 42
# speedup vs baseline: 1.3767x; 1.1360x over previous
"""Causal self-attention Trainium2 kernel (fp8 DoubleRow + natural-AV).

Problem: y = CausalSelfAttention(x) with B=4, T=2048, C=1024, H=16 heads,
head_dim D=64, qkv split order (k, q, v), softmax scale C**-0.5.

Sharding (8 cores): core = 2*b + g  -> batch b in 0..3, head-group g in 0..1
(8 heads per group).  Each core computes, for its batch and its 8 heads:
qkv partial matmuls, causal attention, and the partial output projection
y_partial = att_out @ W_proj[rows of this head group].  The host sums the two
partial projections per batch.

Key speed tricks (tuned against the concourse instruction cost model):
  * k/q QKV matmuls run in fp8(e4m3) with perf_mode=DoubleRow: each
    instruction contracts TWO 128-deep k-tiles at 0.5 cycles/row.
    Weights are pre-scaled by WS=32 on the host so fp8 keeps precision;
    the scale is folded into the softmax exp scale (1/WS^2).
  * S = q^T k runs in fp8 DoubleRow too: D=64 is split into two 32-halves
    stored at different free offsets on quarter partition ranges
    (head h lives on partitions 32*(h%4)..+31).  The host permutes the
    W_qkv columns so the QKV matmul output lands directly in this layout
    (PSUM->SBUF copies stay partition-preserving).
  * AV runs in the natural [q, d] orientation: lhsT = P^T block (exp
    output), rhs = [V_h | ones] so N=65 streamed rows per k-tile instead
    of 512, and causal sparsity is exploited per 128-token q-tile.
    The ones column gives the softmax denominator; a per-partition
    reciprocal + broadcast multiply normalizes.  Four q-tile accumulation
    chains share one PSUM bank (only the first matmul in the bank sets
    start=True; the bank-wide pending-zero covers the other chains).
  * att comes out token-major; PE transpose (identity matmul) flips it to
    channel-major for the bf16 projection.
  * v and proj stay bf16: fp8 there would inject ~2-3% output error.
  * The attention stream is ACT(exp)-bound.  Emission interleaves, at
    S-block granularity: S blocks of chunk X+1, AV chains of chunk X, and
    qkv/v/transpose/proj filler units paced so that emitted PE rows track
    emitted ACT row-equivalents (keeps the PE p-state ramp hot and the
    ACT queue never empty).
"""

import numpy as np
import ml_dtypes

B, T, C, H = 4, 2048, 1024, 16
D = C // H          # 64
HPC = H // 2        # 8 heads per core
CG = C // 2         # 512 channels per head group
P = 128
KT = C // P         # 8 contraction tiles over C
TT = T // P         # 16 token tiles
QC = T // 512       # 4 q chunks of 512
VB = CG // P        # 4 att/channel blocks
WS = 32.0           # fp8 weight pre-scale for k/q

_compiled = {}


def _build(t=T):
    import concourse.bacc as bacc
    import concourse.tile as tile
    import concourse.mybir as mybir

    f32 = mybir.dt.float32
    bf16 = mybir.dt.bfloat16
    f8 = mybir.dt.float8e4
    Exp = mybir.ActivationFunctionType.Exp
    DR = mybir.MatmulPerfMode.DoubleRow

    tt_n = t // P
    qc_n = t // 512
    SCALE = float(C) ** -0.5 / (WS * WS)

    nc = bacc.Bacc("TRN2", target_bir_lowering=False, debug=False,
                   num_devices=8)

    xT_d = nc.dram_tensor("xT", [C, t], bf16, kind="ExternalInput")
    x8_d = nc.dram_tensor("x8T", [C, t], f8, kind="ExternalInput")
    wkq_d = nc.dram_tensor("wkq8", [C, 8, P], f8, kind="ExternalInput")
    wv_d = nc.dram_tensor("wv", [C, CG], bf16, kind="ExternalInput")
    wp_d = nc.dram_tensor("wp", [CG, C], bf16, kind="ExternalInput")
    id_d = nc.dram_tensor("ident", [P, P], bf16, kind="ExternalInput")
    y_d = nc.dram_tensor("y", [C, t], f32, kind="ExternalOutput")

    with tile.TileContext(nc) as tc:
        with (
            tc.tile_pool(name="persist", bufs=1) as persist,
            tc.tile_pool(name="psA", bufs=2, space="PSUM") as psA,
            tc.tile_pool(name="ptP", bufs=16) as ptP,
            tc.tile_pool(name="ptdP", bufs=6) as ptdP,
            tc.tile_pool(name="rcP", bufs=4) as rcP,
            tc.tile_pool(name="atP", bufs=2) as atP,
        ):
            xT = persist.tile([P, KT, t], bf16)
            x8 = persist.tile([P, KT, t], f8)
            wkq_sb = persist.tile([P, KT, 8, P], f8)
            wv_sb = persist.tile([P, KT, CG], bf16)
            wp_sb = persist.tile([P, VB, C], bf16)
            id_sb = persist.tile([P, P], bf16)
            # kq8[32Q+r, kq, s, e, tok] = (x @ Wkq*WS)[tok, 64*(Q+4s)+32e+r]
            kq8 = persist.tile([P, 2, 2, 2, t], f8)
            # engine APs can only start at partition 0/32/64, so the Q=3
            # quarter (partitions 96..127) is DMA-duplicated to base 0 here
            kq8b = persist.tile([32, 2, 2, 2, t], f8)
            v_aug = persist.tile([P, tt_n, HPC, D + 1], bf16)
            att_nat = persist.tile([P, tt_n, HPC, D], bf16)
            ybuf = persist.tile([P, KT, 512], f32)

            # PE warm-up: dependency-free matmuls run during the input-DMA
            # window so the p-state ramp is hot when real work starts.
            wu_a = persist.tile([P, P], bf16)
            wu_b = persist.tile([P, 512], bf16)
            nc.vector.memset(wu_a, 0.0)
            nc.vector.memset(wu_b, 0.0)
            for _ in range(12):
                wps = psA.tile([P, 512], f32, name="wups", tag="qp", bufs=2)
                nc.tensor.matmul(wps, lhsT=wu_a, rhs=wu_b,
                                 start=True, stop=True,
                                 skip_group_check=True)

            # ---- input loads, ordered so token-chunk 0 lands first ----
            wkq_r = wkq_d.ap().rearrange("(kt p) b m -> p kt b m", p=P)
            x8_r = x8_d.ap().rearrange("(kt p) n -> p kt n", p=P)
            xT_r = xT_d.ap().rearrange("(kt p) n -> p kt n", p=P)
            wv_r = wv_d.ap().rearrange("(kt p) m -> p kt m", p=P)
            wp_r = wp_d.ap().rearrange("(kt p) m -> p kt m", p=P)
            nc.sync.dma_start(wkq_sb[:, :, 0:4, :], wkq_r[:, :, 0:4, :])
            nc.sync.dma_start(x8[:, :, 0:512], x8_r[:, :, 0:512])
            nc.sync.dma_start(wkq_sb[:, :, 4:8, :], wkq_r[:, :, 4:8, :])
            nc.sync.dma_start(xT[:, :, 0:512], xT_r[:, :, 0:512])
            nc.sync.dma_start(wv_sb, wv_r)
            nc.sync.dma_start(id_sb, id_d.ap())
            # only the ones-column needs initializing; v units fill 0:D
            nc.vector.memset(v_aug[:, :, :, D:D + 1], 1.0)

            def emit_load(c):
                """Stream token chunk c of x8/xT (and wp alongside c=1);
                lazy so the DMA engines stay available for the kq8b
                fix-up transfers during the early attention chunks."""
                lo, hi = c * 512, (c + 1) * 512
                nc.sync.dma_start(x8[:, :, lo:hi], x8_r[:, :, lo:hi])
                nc.sync.dma_start(xT[:, :, lo:hi], xT_r[:, :, lo:hi])
                if c == 1:
                    nc.sync.dma_start(wp_sb, wp_r)

            # ---- pacing state ----
            st8 = {"pe": 0, "act": 0}

            def pe(rows):
                st8["pe"] += rows

            def act(rows):
                st8["act"] += rows

            # ---- unit emitters ----
            def emit_kq_unit(kq, s, e, c):
                """One 128-out-channel block of k or q for one 512-tok
                chunk, fp8 DoubleRow over kt pairs."""
                blk = 4 * s + 2 * kq + e
                ps = psA.tile([P, 512], f32, name="ps", tag="qp", bufs=2)
                for kt in range(4):
                    nc.tensor.matmul(
                        ps,
                        lhsT=wkq_sb[:, 2 * kt:2 * kt + 2, blk, :],
                        rhs=x8[:, 2 * kt:2 * kt + 2, c * 512:(c + 1) * 512],
                        start=(kt == 0), stop=(kt == 3),
                        perf_mode=DR, skip_group_check=True)
                nc.vector.tensor_copy(
                    kq8[:, kq, s, e, c * 512:(c + 1) * 512], ps)
                nc.sync.dma_start(
                    kq8b[:, kq, s, e, c * 512:(c + 1) * 512],
                    kq8[96:128, kq, s, e, c * 512:(c + 1) * 512])
                pe(1024)

            def emit_v_unit(tt, hp):
                """v for one 128-token tile and one head pair (natural
                layout, bf16).  Head-pair granularity keeps the forced
                emission ahead of AV small enough to hide under the
                exp stream."""
                ps = psA.tile([P, 2 * D], f32, name="psv", tag="qp", bufs=2)
                for kt in range(KT):
                    nc.tensor.matmul(
                        ps,
                        lhsT=xT[:, kt, tt * P:(tt + 1) * P],
                        rhs=wv_sb[:, kt, 2 * hp * D:(2 * hp + 2) * D],
                        start=(kt == 0), stop=(kt == KT - 1),
                        skip_group_check=True)
                nc.vector.tensor_copy(
                    v_aug[:, tt, 2 * hp:2 * hp + 2, 0:D],
                    ps.rearrange("p (h d) -> p h d", d=D))
                pe(1024)

            def emit_transpose(tt, cb, attTc):
                """att_nat[:, tt, heads 2cb..2cb+1] -> attTc channel-major."""
                pst = psA.tile([P, P], bf16, name="pst", tag="qp", bufs=2)
                nc.tensor.transpose(
                    pst, att_nat[:, tt, 2 * cb:2 * cb + 2, :], id_sb)
                nc.vector.tensor_copy(attTc[:, cb, tt % 4, :], pst)
                pe(128)

            def emit_proj_unit(mb, c, attTc):
                ps = psA.tile([P, 512], f32, name="psp", tag="qp", bufs=2)
                for kt in range(VB):
                    nc.tensor.matmul(
                        ps,
                        lhsT=wp_sb[:, kt, mb * P:(mb + 1) * P],
                        rhs=attTc[:, kt, :, :],
                        start=(kt == 0), stop=(kt == VB - 1),
                        skip_group_check=True)
                nc.vector.tensor_copy(ybuf[:, mb, :], ps)
                nc.sync.dma_start(
                    y_d[mb * P:(mb + 1) * P, c * 512:(c + 1) * 512],
                    ybuf[:, mb, :])
                pe(2048)

            # ---- filler queues ----
            # prio: transposes/proj (drain ASAP, ahead of pacing)
            # fills: kq/v units, paced against the ACT stream
            prio = []
            fills = []  # (fn, key); key = ("kq", c, s) / ("v"|"ld", i, _)
            for c in range(qc_n):
                if c > 0:
                    fills.append(((lambda c=c: emit_load(c)), ("ld", c, 0)))
                for s in (0, 1):
                    for kq in (0, 1):
                        for e in (0, 1):
                            fills.append(
                                ((lambda kq=kq, s=s, e=e, c=c:
                                  emit_kq_unit(kq, s, e, c)),
                                 ("kq", c, s)))
                for hp in (0, 2, 1, 3):
                    for tt in range(4 * c, 4 * c + 4):
                        fills.append(
                            ((lambda tt=tt, hp=hp: emit_v_unit(tt, hp)),
                             ("v", tt, hp)))

            LEAD = 6000  # rows of PE work kept queued ahead of ACT
            st8["dummies"] = 0

            def emit_fills():
                """Priority units first (up to 3 per call), then top up
                pacing fillers until emitted PE rows lead ACT rows.  When
                everything is drained but the PE is behind the ACT stream,
                emit keep-warm matmuls so the p-state ramp stays hot."""
                n = 0
                while prio and n < 3:
                    prio.pop(0)()
                    n += 1
                while fills and st8["pe"] < st8["act"] + LEAD:
                    fn, key = fills.pop(0)
                    fn()
                while (not fills and not prio and st8["dummies"] < 200
                       and st8["pe"] < st8["act"]):
                    wps = psA.tile([P, 512], f32, name="wrm", tag="qp",
                                   bufs=2)
                    nc.tensor.matmul(wps, lhsT=wu_a, rhs=wu_b,
                                     start=True, stop=True,
                                     skip_group_check=True)
                    pe(512)
                    st8["dummies"] += 1

            def force_ld(c):
                i = 0
                while i < len(fills):
                    fn, key = fills[i]
                    if key[0] == "ld" and key[1] <= c:
                        fills.pop(i)
                        fn()
                    else:
                        i += 1

            def force_kq(c, s):
                force_ld(c)
                i = 0
                while i < len(fills):
                    fn, key = fills[i]
                    if key[0] == "kq" and key[1] <= c and key[2] == s:
                        fills.pop(i)
                        fn()
                    else:
                        i += 1

            def force_v(tt, hp):
                force_ld(tt // 4)
                i = 0
                while i < len(fills):
                    fn, key = fills[i]
                    if key[0] == "v" and key[1] <= tt and key[2] == hp:
                        fills.pop(i)
                        fn()
                    else:
                        i += 1

            # ---- attention building blocks ----
            def s_block_emitters(hp, c):
                """Per-block closures for S^T + exp of head pair hp,
                chunk c; returns (emitters, pts) where pts is filled in
                as blocks run."""
                s_h = hp // 2
                q0, q1 = (2 * hp) % 4, (2 * hp + 1) % 4
                nfull = 4 * c
                pts = []
                ems = []

                def kq_op(kq, Q, c0, c1):
                    if Q == 3:
                        return kq8b[:, kq, s_h, :, c0:c1]
                    return kq8[32 * Q:32 * Q + 32, kq, s_h, :, c0:c1]

                def full_block(j):
                    def em():
                        st = psA.tile([P, 2, 512], f32, name="st", tag="st",
                                      bufs=2)
                        for hi, Q in ((0, q0), (1, q1)):
                            nc.tensor.matmul(
                                st[:, hi, :],
                                lhsT=kq_op(0, Q, j * P, (j + 1) * P),
                                rhs=kq_op(1, Q, c * 512, (c + 1) * 512),
                                start=True, stop=True,
                                perf_mode=DR, skip_group_check=True)
                        pt = ptP.tile([P, 2, 512], bf16, name="pt",
                                      tag="pt")
                        nc.scalar.activation(pt, st, Exp, scale=SCALE)
                        pts.append((pt, j, 0))
                        pe(512)
                        act(2 * 1024 + 444)
                    return em

                def diag_block(dj):
                    def em():
                        j = nfull + dj
                        off = P * dj
                        w = 512 - off
                        st = psA.tile([P, 2, 512], f32, name="std",
                                      tag="st")
                        for hi, Q in ((0, q0), (1, q1)):
                            nc.tensor.matmul(
                                st[:, hi, 0:w],
                                lhsT=kq_op(0, Q, j * P, (j + 1) * P),
                                rhs=kq_op(1, Q, c * 512 + off,
                                          (c + 1) * 512),
                                start=True, stop=True,
                                perf_mode=DR, skip_group_check=True)
                        pt = ptdP.tile([P, 2, 512], bf16, name="ptd",
                                       tag="ptd")
                        nc.scalar.activation(pt[:, :, 0:w], st[:, :, 0:w],
                                             Exp, scale=SCALE)
                        nc.gpsimd.affine_select(
                            pt[:, :, 0:P], pt[:, :, 0:P],
                            pattern=[[0, 2], [1, P]],
                            compare_op=mybir.AluOpType.is_ge,
                            fill=0.0, base=0, channel_multiplier=-1)
                        pts.append((pt, j, off))
                        pe(w)
                        act(4 * w + 444)
                    return em

                for j in range(nfull):
                    ems.append(full_block(j))
                for dj in range(4):
                    ems.append(diag_block(dj))
                return ems, pts

            def av_emitters(hp, c, pts):
                """Per-head closures: 4 accumulation chains + drain."""
                ems = []

                def head(hi):
                    def em():
                        h = 2 * hp + hi
                        avp = psA.tile([P, 4, P], f32, name=f"avp{hi}",
                                       tag="avp", bufs=2)
                        rows = 0
                        for u in range(4):
                            tq = 4 * c + u
                            chain = [pj for pj in pts if pj[1] <= tq]
                            n = len(chain)
                            for idx, (pt, j, off) in enumerate(chain):
                                lo = u * P - off
                                nc.tensor.matmul(
                                    avp[:, u, 0:D + 1],
                                    lhsT=pt[:, hi, lo:lo + P],
                                    rhs=v_aug[:, j, h, :],
                                    start=(u == 0 and idx == 0),
                                    stop=(idx == n - 1),
                                    skip_group_check=True)
                                rows += D + 1
                        rc = rcP.tile([P, 4, 1], f32, name="rc", tag="rc",
                                      bufs=4)
                        nc.vector.reciprocal(rc, avp[:, :, D:D + 1])
                        nc.vector.tensor_mul(
                            att_nat[:, 4 * c:4 * c + 4, h, :],
                            avp[:, :, 0:D],
                            rc.broadcast_to([P, 4, D]))
                        pe(rows)
                    return em

                return [head(0), head(1)]

            # ---- main interleaved emission ----
            # hp order (0,2,1,3): the first chunks use heads on quarters
            # 0..2 only, so nothing waits on the kq8b fix-up DMAs while the
            # input-load train still occupies the DMA engines.
            # c order (1,2,3,0): the epilogue chunk is the small c=0 one,
            # so little work trails the final exp
            c_order = list(range(qc_n))
            chunk_seq = [(hp, c) for c in c_order for hp in (0, 2, 1, 3)]
            done_hp = set()
            pending = None   # (hp, c, av emitter list)

            attTc_map = {}

            def queue_transposes(pc, cb):
                """Transposes of head pair cb for token chunk pc -> prio,
                as soon as that pair's AV drains."""
                if pc not in attTc_map:
                    attTc_map[pc] = atP.tile([P, VB, 4, P], bf16,
                                             name="attTc", tag="attTc",
                                             bufs=2)
                a = attTc_map[pc]
                for tt in range(4 * pc, 4 * pc + 4):
                    prio.append((lambda tt=tt, cb=cb, a=a:
                                 emit_transpose(tt, cb, a)))

            def queue_proj(pc):
                a = attTc_map.pop(pc)
                for mb in range(KT):
                    prio.append((lambda mb=mb, pc=pc, a=a:
                                 emit_proj_unit(mb, pc, a)))

            for hp, c in chunk_seq:
                force_kq(c, hp // 2)
                s_ems, pts = s_block_emitters(hp, c)
                avq = []
                if pending is not None:
                    php, pc, avq = pending
                    force_v(4 * pc + 3, php)
                for bi, em in enumerate(s_ems):
                    em()
                    if bi >= 2 and avq:
                        avq.pop(0)()
                    emit_fills()
                while avq:
                    avq.pop(0)()
                    emit_fills()
                if pending is not None:
                    queue_transposes(pc, php)
                    done_hp.add((php, pc))
                    if all((h2, pc) in done_hp for h2 in range(4)):
                        queue_proj(pc)
                pending = (hp, c, av_emitters(hp, c, pts))
                if hp == 3 and c + 1 < qc_n:
                    # pull the next chunk's s=0 kq units (and loads) in
                    # under this chunk's large exp cover
                    force_kq(c + 1, 0)

            php, pc, avq = pending
            force_v(4 * pc + 3, php)
            for em in avq:
                em()
                while prio:
                    prio.pop(0)()
            queue_transposes(pc, php)
            while prio:
                prio.pop(0)()
            queue_proj(pc)
            while prio:
                prio.pop(0)()
            while fills:
                fn, key = fills.pop(0)
                fn()

    nc.compile()
    return nc


def _get_compiled(t=T):
    if t not in _compiled:
        _compiled[t] = _build(t)
    return _compiled[t]


def make_in_maps(x, W_qkv, W_proj):
    bf = ml_dtypes.bfloat16
    f8 = ml_dtypes.float8_e4m3
    x = np.asarray(x, dtype=np.float32)
    W_qkv = np.asarray(W_qkv, dtype=np.float32)
    W_proj = np.asarray(W_proj, dtype=np.float32)
    ident = np.eye(P, dtype=np.float32).astype(bf)
    in_maps = []
    for core in range(8):
        b, g = core // 2, core % 2
        xT = np.ascontiguousarray(x[b].T)           # [C, T]
        Wk = W_qkv[:, g * CG:(g + 1) * CG]
        Wq = W_qkv[:, C + g * CG:C + (g + 1) * CG]
        Wv = W_qkv[:, 2 * C + g * CG:2 * C + (g + 1) * CG]
        # permute k/q columns into the quarter layout:
        # block (kq, s, e) partition 32Q+r <- column 64*(Q+4s)+32e+r
        wkq8 = np.empty((C, 8, P), dtype=np.float32)
        for kqi, W in ((0, Wk), (1, Wq)):
            Wr = (W * WS).reshape(C, 2, 4, 2, 32)   # [C, s, Q, e, r]
            for s in range(2):
                for e in range(2):
                    blk = 4 * s + 2 * kqi + e
                    wkq8[:, blk, :] = Wr[:, s, :, e, :].reshape(C, P)
        in_maps.append({
            "xT": xT.astype(bf),
            "x8T": xT.astype(f8),
            "wkq8": wkq8.astype(f8),
            "wv": np.ascontiguousarray(Wv).astype(bf),
            "wp": np.ascontiguousarray(
                W_proj[g * CG:(g + 1) * CG, :]).astype(bf),
            "ident": ident,
        })
    return in_maps


def _run_axon_nodonate(nc, in_maps, n_cores=8):
    """Execute via PJRT/shard_map WITHOUT output-buffer donation.

    bass2jax.run_bass_via_pjrt donates the zero output operands; under the
    axon transport that donation intermittently corrupts multi-core results.
    This kernel writes every element of its output, so donation is not
    needed for correctness -- pass non-donated zero operands instead.
    """
    import jax
    from jax.sharding import Mesh, PartitionSpec
    from jax.experimental.shard_map import shard_map
    import concourse.mybir as mybir
    from concourse.bass2jax import _bass_exec_p, install_neuronx_cc_hook

    install_neuronx_cc_hook()
    in_names, out_names, out_avals = [], [], []
    for alloc in nc.m.functions[0].allocations:
        if not isinstance(alloc, mybir.MemoryLocationSet):
            continue
        name = alloc.memorylocations[0].name
        if alloc.kind == "ExternalInput":
            in_names.append(name)
        elif alloc.kind == "ExternalOutput":
            out_names.append(name)
            out_avals.append(jax.core.ShapedArray(
                tuple(alloc.tensor_shape), mybir.dt.np(alloc.dtype)))
    n_params = len(in_names)
    all_names = in_names + out_names
    pid_name = nc.partition_id_tensor.name if nc.partition_id_tensor else None

    def _body(*args):
        return tuple(_bass_exec_p.bind(
            *args,
            out_avals=tuple(out_avals),
            in_names=tuple(all_names),
            out_names=tuple(out_names),
            lowering_input_output_aliases=(),
            sim_require_finite=True,
            sim_require_nnan=True,
            nc=nc,
        ))

    devices = jax.devices()[:n_cores]
    mesh = Mesh(np.asarray(devices), ("core",))
    fn = jax.jit(
        shard_map(_body, mesh=mesh,
                  in_specs=(PartitionSpec("core"),) * (n_params + len(out_names)),
                  out_specs=(PartitionSpec("core"),) * len(out_names),
                  check_rep=False),
        keep_unused=True)
    concat_in = [
        np.concatenate([
            np.asarray(in_maps[c].get(
                nm, np.array([[c]], dtype=np.uint32) if nm == pid_name
                else None))
            for c in range(n_cores)], 0)
        for nm in in_names
    ]
    concat_zeros = [
        np.zeros((n_cores * a.shape[0], *a.shape[1:]), a.dtype)
        for a in out_avals
    ]
    out = fn(*concat_in, *concat_zeros)
    return [
        {nm: np.asarray(out[i]).reshape(n_cores, *out_avals[i].shape)[c]
         for i, nm in enumerate(out_names)}
        for c in range(n_cores)
    ]


def kernel(x, W_qkv, W_proj, _trace=False):
    from concourse._compat import axon_active

    nc = _get_compiled()
    in_maps = make_in_maps(x, W_qkv, W_proj)
    if axon_active():
        results = _run_axon_nodonate(nc, in_maps)
    else:
        import concourse.bass_utils as bass_utils
        res = bass_utils.run_bass_kernel_spmd(
            nc, in_maps, core_ids=list(range(8)), trace=_trace)
        if _trace:
            kernel.last_results = res
        results = res.results
    y = np.zeros((B, T, C), np.float32)
    for core in range(8):
        y[core // 2] += results[core]["y"].T
    return y


# revision 61
# speedup vs baseline: 1.3870x; 1.0075x over previous
"""Causal self-attention Trainium2 kernel (fp8 DoubleRow + natural-AV).

Problem: y = CausalSelfAttention(x) with B=4, T=2048, C=1024, H=16 heads,
head_dim D=64, qkv split order (k, q, v), softmax scale C**-0.5.

Sharding (8 cores): core = 2*b + g  -> batch b in 0..3, head-group g in 0..1
(8 heads per group).  Each core computes, for its batch and its 8 heads:
qkv partial matmuls, causal attention, and the partial output projection
y_partial = att_out @ W_proj[rows of this head group].  The host sums the two
partial projections per batch.

Key speed tricks (tuned against the concourse instruction cost model):
  * k/q QKV matmuls run in fp8(e4m3) with perf_mode=DoubleRow: each
    instruction contracts TWO 128-deep k-tiles at 0.5 cycles/row.
    Weights are pre-scaled by WS=32 on the host so fp8 keeps precision;
    the scale is folded into the softmax exp scale (1/WS^2).
  * S = q^T k runs in fp8 DoubleRow too: D=64 is split into two 32-halves
    stored at different free offsets on quarter partition ranges
    (head h lives on partitions 32*(h%4)..+31).  The host permutes the
    W_qkv columns so the QKV matmul output lands directly in this layout
    (PSUM->SBUF copies stay partition-preserving).
  * AV runs in the natural [q, d] orientation: lhsT = P^T block (exp
    output), rhs = [V_h | ones] so N=65 streamed rows per k-tile instead
    of 512, and causal sparsity is exploited per 128-token q-tile.
    The ones column gives the softmax denominator; a per-partition
    reciprocal + broadcast multiply normalizes.  Four q-tile accumulation
    chains share one PSUM bank (only the first matmul in the bank sets
    start=True; the bank-wide pending-zero covers the other chains).
  * att comes out token-major; PE transpose (identity matmul) flips it to
    channel-major for the bf16 projection.
  * v and proj stay bf16: fp8 there would inject ~2-3% output error.
  * The attention stream is ACT(exp)-bound.  Emission interleaves, at
    S-block granularity: S blocks of chunk X+1, AV chains of chunk X, and
    qkv/v/transpose/proj filler units paced so that emitted PE rows track
    emitted ACT row-equivalents (keeps the PE p-state ramp hot and the
    ACT queue never empty).
"""

import numpy as np
import ml_dtypes

B, T, C, H = 4, 2048, 1024, 16
D = C // H          # 64
HPC = H // 2        # 8 heads per core
CG = C // 2         # 512 channels per head group
P = 128
KT = C // P         # 8 contraction tiles over C
TT = T // P         # 16 token tiles
QC = T // 512       # 4 q chunks of 512
VB = CG // P        # 4 att/channel blocks
WS = 32.0           # fp8 weight pre-scale for k/q

_compiled = {}


def _build(t=T):
    import concourse.bacc as bacc
    import concourse.tile as tile
    import concourse.mybir as mybir

    f32 = mybir.dt.float32
    bf16 = mybir.dt.bfloat16
    f8 = mybir.dt.float8e4
    Exp = mybir.ActivationFunctionType.Exp
    DR = mybir.MatmulPerfMode.DoubleRow

    tt_n = t // P
    qc_n = t // 512
    SCALE = float(C) ** -0.5 / (WS * WS)

    nc = bacc.Bacc("TRN2", target_bir_lowering=False, debug=False,
                   num_devices=8)

    xT_d = nc.dram_tensor("xT", [C, t], bf16, kind="ExternalInput")
    x8_d = nc.dram_tensor("x8T", [C, t], f8, kind="ExternalInput")
    wkq_d = nc.dram_tensor("wkq8", [C, C], f8, kind="ExternalInput")
    wv_d = nc.dram_tensor("wv", [C, CG], bf16, kind="ExternalInput")
    wp_d = nc.dram_tensor("wp", [CG, C], bf16, kind="ExternalInput")
    id_d = nc.dram_tensor("ident", [P, P], bf16, kind="ExternalInput")
    y_d = nc.dram_tensor("y", [C, t], f32, kind="ExternalOutput")

    with tile.TileContext(nc) as tc:
        with (
            tc.tile_pool(name="persist", bufs=1) as persist,
            tc.tile_pool(name="psA", bufs=2, space="PSUM") as psA,
            tc.tile_pool(name="ptP", bufs=18) as ptP,
            tc.tile_pool(name="ptdP", bufs=8) as ptdP,
            tc.tile_pool(name="rcP", bufs=4) as rcP,
            tc.tile_pool(name="atP", bufs=2) as atP,
        ):
            xT = persist.tile([P, KT, t], bf16)
            x8 = persist.tile([P, KT, t], f8)
            wkq_sb = persist.tile([P, KT, C], f8)
            wv_sb = persist.tile([P, KT, CG], bf16)
            wp_sb = persist.tile([P, VB, C], bf16)
            id_sb = persist.tile([P, P], bf16)
            # k/q activations in fp8, laid out so every S operand starts
            # at partition 0/32/64 (engine AP bases cannot encode 96):
            #  kq8a[32Q+r, kq, s, e, tok] = slot-head Q in {0,1,2}
            #  kq8d[32s+r, kq, e, tok]    = slot-head 3 of slot s
            kq8a = persist.tile([96, 2, 2, 2, t], f8)
            kq8d = persist.tile([64, 2, 2, t], f8)
            v_aug = persist.tile([P, tt_n, HPC, D + 1], bf16)
            att_nat = persist.tile([P, tt_n, HPC, D], bf16)
            ybuf = persist.tile([P, KT, 512], f32)

            # PE warm-up: dependency-free matmuls run during the input-DMA
            # window so the p-state ramp is hot when real work starts.
            wu_a = persist.tile([P, P], bf16)
            wu_b = persist.tile([P, 512], bf16)
            nc.vector.memset(wu_a, 0.0)
            nc.vector.memset(wu_b, 0.0)
            for _ in range(12):
                wps = psA.tile([P, 512], f32, name="wups", tag="qp", bufs=2)
                nc.tensor.matmul(wps, lhsT=wu_a, rhs=wu_b,
                                 start=True, stop=True,
                                 skip_group_check=True)

            # ---- input loads, ordered so token-chunk 0 lands first ----
            wkq_r = wkq_d.ap().rearrange("(kt p) m -> p kt m", p=P)
            x8_r = x8_d.ap().rearrange("(kt p) n -> p kt n", p=P)
            xT_r = xT_d.ap().rearrange("(kt p) n -> p kt n", p=P)
            wv_r = wv_d.ap().rearrange("(kt p) m -> p kt m", p=P)
            wp_r = wp_d.ap().rearrange("(kt p) m -> p kt m", p=P)
            nc.sync.dma_start(wkq_sb[:, :, 0:384], wkq_r[:, :, 0:384])
            nc.sync.dma_start(x8[:, :, 0:512], x8_r[:, :, 0:512])
            nc.sync.dma_start(wkq_sb[:, :, 384:C], wkq_r[:, :, 384:C])
            nc.sync.dma_start(wv_sb[:, :, 0:2 * D], wv_r[:, :, 0:2 * D])
            nc.sync.dma_start(xT[:, :, 0:512], xT_r[:, :, 0:512])
            nc.sync.dma_start(id_sb, id_d.ap())

            def emit_wv_load(hp):
                lo = 2 * hp * D
                nc.sync.dma_start(wv_sb[:, :, lo:lo + 2 * D],
                                  wv_r[:, :, lo:lo + 2 * D])
            # only the ones-column needs initializing; v units fill 0:D
            nc.vector.memset(v_aug[:, :, :, D:D + 1], 1.0)

            def emit_load(c):
                """Stream token chunk c of x8/xT (and wp alongside c=1);
                lazy so the DMA engines stay available for the kq8b
                fix-up transfers during the early attention chunks."""
                lo, hi = c * 512, (c + 1) * 512
                nc.sync.dma_start(x8[:, :, lo:hi], x8_r[:, :, lo:hi])
                nc.sync.dma_start(xT[:, :, lo:hi], xT_r[:, :, lo:hi])
                if c == 1:
                    nc.sync.dma_start(wp_sb, wp_r)

            # ---- pacing state ----
            st8 = {"pe": 0, "act": 0}
            import kernel as _km
            _km._dbg_map = {}

            def _dbg(inst, label):
                try:
                    _km._dbg_map[inst.inst.name] = label
                except Exception:
                    try:
                        _km._dbg_map[inst.name] = label
                    except Exception:
                        pass

            def pe(rows):
                st8["pe"] += rows

            def act(rows):
                st8["act"] += rows

            # ---- unit emitters ----
            # wkq column layout (1024 cols):
            #   [384s + 96a, +96): A-block (s, a=2kq+e), slot-heads 0..2
            #   [768 + 64d, +64):  D-block d=2kq+e: head 3 of s0 (32),
            #                      then head 3 of s1 (32)
            def emit_kq_unit(kq, s, e, c):
                """One A-block (M=96, slot-heads 0..2) of k or q for one
                512-tok chunk, fp8 DoubleRow over kt pairs."""
                a = 2 * kq + e
                lo = 384 * s + 96 * a
                ps = psA.tile([96, 512], f32, name="ps", tag="qp", bufs=2)
                for kt in range(4):
                    nc.tensor.matmul(
                        ps,
                        lhsT=wkq_sb[:, 2 * kt:2 * kt + 2, lo:lo + 96],
                        rhs=x8[:, 2 * kt:2 * kt + 2, c * 512:(c + 1) * 512],
                        start=(kt == 0), stop=(kt == 3),
                        perf_mode=DR, skip_group_check=True)
                nc.vector.tensor_copy(
                    kq8a[:, kq, s, e, c * 512:(c + 1) * 512], ps)
                pe(1024)

            def emit_kqd_unit(kq, e, c):
                """One D-block (M=64, both slots' head 3) for one
                512-tok chunk."""
                d = 2 * kq + e
                lo = 768 + 64 * d
                ps = psA.tile([64, 512], f32, name="psd", tag="qp", bufs=2)
                for kt in range(4):
                    nc.tensor.matmul(
                        ps,
                        lhsT=wkq_sb[:, 2 * kt:2 * kt + 2, lo:lo + 64],
                        rhs=x8[:, 2 * kt:2 * kt + 2, c * 512:(c + 1) * 512],
                        start=(kt == 0), stop=(kt == 3),
                        perf_mode=DR, skip_group_check=True)
                nc.vector.tensor_copy(
                    kq8d[:, kq, e, c * 512:(c + 1) * 512], ps)
                pe(1024)

            def emit_v_unit(tt, hp):
                """v for one 128-token tile and one head pair (natural
                layout, bf16).  Head-pair granularity keeps the forced
                emission ahead of AV small enough to hide under the
                exp stream."""
                ps = psA.tile([P, 2 * D], f32, name="psv", tag="qp", bufs=2)
                for kt in range(KT):
                    nc.tensor.matmul(
                        ps,
                        lhsT=xT[:, kt, tt * P:(tt + 1) * P],
                        rhs=wv_sb[:, kt, 2 * hp * D:(2 * hp + 2) * D],
                        start=(kt == 0), stop=(kt == KT - 1),
                        skip_group_check=True)
                nc.vector.tensor_copy(
                    v_aug[:, tt, 2 * hp:2 * hp + 2, 0:D],
                    ps.rearrange("p (h d) -> p h d", d=D))
                pe(1024)

            def emit_transpose(tt, cb, attTc):
                """att_nat[:, tt, heads 2cb..2cb+1] -> attTc channel-major."""
                pst = psA.tile([P, P], bf16, name="pst", tag="qp", bufs=2)
                nc.tensor.transpose(
                    pst, att_nat[:, tt, 2 * cb:2 * cb + 2, :], id_sb)
                nc.vector.tensor_copy(attTc[:, cb, tt % 4, :], pst)
                pe(128)

            def emit_proj_unit(mb, c, attTc, tag="qp"):
                ps = psA.tile([P, 512], f32, name="psp", tag=tag, bufs=2)
                for kt in range(VB):
                    nc.tensor.matmul(
                        ps,
                        lhsT=wp_sb[:, kt, mb * P:(mb + 1) * P],
                        rhs=attTc[:, kt, :, :],
                        start=(kt == 0), stop=(kt == VB - 1),
                        skip_group_check=True)
                nc.vector.tensor_copy(ybuf[:, mb, :], ps)
                nc.sync.dma_start(
                    y_d[mb * P:(mb + 1) * P, c * 512:(c + 1) * 512],
                    ybuf[:, mb, :])
                pe(2048)

            # ---- filler queues ----
            # prio: transposes/proj (drain ASAP, ahead of pacing)
            # fills: kq/v units, paced against the ACT stream
            prio = []
            fills = []  # (fn, key); key = ("kq", c, s) / ("v"|"ld", i, _)
            for hp in (2, 1, 3):
                fills.append(((lambda hp=hp: emit_wv_load(hp)),
                              ("lw", hp, 0)))
            for c in range(qc_n):
                if c > 0:
                    fills.append(((lambda c=c: emit_load(c)), ("ld", c, 0)))
                for s in (0, 1):
                    for kq in (0, 1):
                        for e in (0, 1):
                            fills.append(
                                ((lambda kq=kq, s=s, e=e, c=c:
                                  emit_kq_unit(kq, s, e, c)),
                                 ("kq", c, s)))
                for kq in (0, 1):
                    for e in (0, 1):
                        fills.append(
                            ((lambda kq=kq, e=e, c=c:
                              emit_kqd_unit(kq, e, c)), ("kq", c, 0)))
                for hp in (0, 2, 1, 3):
                    for tt in range(4 * c, 4 * c + 4):
                        fills.append(
                            ((lambda tt=tt, hp=hp: emit_v_unit(tt, hp)),
                             ("v", tt, hp)))

            LEAD = 6000  # rows of PE work kept queued ahead of ACT
            st8["dummies"] = 0

            def emit_fills():
                """Priority units first (up to 3 per call), then top up
                pacing fillers until emitted PE rows lead ACT rows.  When
                everything is drained but the PE is behind the ACT stream,
                emit keep-warm matmuls so the p-state ramp stays hot."""
                n = 0
                while prio and n < 3:
                    prio.pop(0)()
                    n += 1
                while fills and st8["pe"] < st8["act"] + LEAD:
                    fn, key = fills.pop(0)
                    fn()
                while (not fills and not prio and st8["dummies"] < 200
                       and st8["pe"] < st8["act"]):
                    wps = psA.tile([P, 512], f32, name="wrm", tag="qp",
                                   bufs=2)
                    nc.tensor.matmul(wps, lhsT=wu_a, rhs=wu_b,
                                     start=True, stop=True,
                                     skip_group_check=True)
                    pe(512)
                    st8["dummies"] += 1

            def force_ld(c):
                i = 0
                while i < len(fills):
                    fn, key = fills[i]
                    if key[0] == "ld" and key[1] <= c:
                        fills.pop(i)
                        fn()
                    else:
                        i += 1

            def force_kq(c, s):
                force_ld(c)
                i = 0
                while i < len(fills):
                    fn, key = fills[i]
                    if key[0] == "kq" and key[1] <= c and key[2] == s:
                        fills.pop(i)
                        fn()
                    else:
                        i += 1

            def force_v(tt, hp):
                force_ld(tt // 4)
                i = 0
                while i < len(fills):
                    fn, key = fills[i]
                    if ((key[0] == "v" and key[1] <= tt and key[2] == hp)
                            or (key[0] == "lw" and key[1] == hp)):
                        fills.pop(i)
                        fn()
                    else:
                        i += 1

            # ---- attention building blocks ----
            def s_block_emitters(hp, c):
                """Per-block closures for S^T + exp of head pair hp,
                chunk c; returns (emitters, pts) where pts is filled in
                as blocks run."""
                s_h = hp // 2
                q0, q1 = (2 * hp) % 4, (2 * hp + 1) % 4
                nfull = 4 * c
                pts = []
                ems = []

                def kq_op(kq, Q, c0, c1):
                    if Q == 3:
                        lo = 32 * s_h
                        return kq8d[lo:lo + 32, kq, :, c0:c1]
                    return kq8a[32 * Q:32 * Q + 32, kq, s_h, :, c0:c1]

                def full_block(j):
                    def em():
                        st = psA.tile([P, 2, 512], f32, name="st", tag="st",
                                      bufs=2)
                        for hi, Q in ((0, q0), (1, q1)):
                            nc.tensor.matmul(
                                st[:, hi, :],
                                lhsT=kq_op(0, Q, j * P, (j + 1) * P),
                                rhs=kq_op(1, Q, c * 512, (c + 1) * 512),
                                start=True, stop=True,
                                perf_mode=DR, skip_group_check=True)
                        pt = ptP.tile([P, 2, 512], bf16, name="pt",
                                      tag="pt")
                        _i = nc.scalar.activation(pt, st, Exp, scale=SCALE)
                        _dbg(_i, f"exp S({hp},{c})j{j}")
                        pts.append((pt, j, 0))
                        pe(512)
                        act(2 * 1024 + 444)
                    return em

                def diag_block(dj):
                    def em():
                        j = nfull + dj
                        off = P * dj
                        w = 512 - off
                        st = psA.tile([P, 2, 512], f32, name="std",
                                      tag="st")
                        for hi, Q in ((0, q0), (1, q1)):
                            nc.tensor.matmul(
                                st[:, hi, 0:w],
                                lhsT=kq_op(0, Q, j * P, (j + 1) * P),
                                rhs=kq_op(1, Q, c * 512 + off,
                                          (c + 1) * 512),
                                start=True, stop=True,
                                perf_mode=DR, skip_group_check=True)
                        pt = ptdP.tile([P, 2, 512], bf16, name="ptd",
                                       tag="ptd")
                        _i = nc.scalar.activation(pt[:, :, 0:w],
                                                  st[:, :, 0:w],
                                                  Exp, scale=SCALE)
                        _dbg(_i, f"exp S({hp},{c})d{dj}")
                        nc.gpsimd.affine_select(
                            pt[:, :, 0:P], pt[:, :, 0:P],
                            pattern=[[0, 2], [1, P]],
                            compare_op=mybir.AluOpType.is_ge,
                            fill=0.0, base=0, channel_multiplier=-1)
                        pts.append((pt, j, off))
                        pe(w)
                        act(4 * w + 444)
                    return em

                for j in range(nfull):
                    ems.append(full_block(j))
                for dj in range(4):
                    ems.append(diag_block(dj))
                return ems, pts

            def av_emitters(hp, c, pts):
                """Per-head closures: 4 accumulation chains + drain."""
                ems = []

                def head(hi):
                    def em():
                        h = 2 * hp + hi
                        avp = psA.tile([P, 4, P], f32, name=f"avp{hi}",
                                       tag="avp", bufs=2)
                        rows = 0
                        for u in range(4):
                            tq = 4 * c + u
                            chain = [pj for pj in pts if pj[1] <= tq]
                            n = len(chain)
                            for idx, (pt, j, off) in enumerate(chain):
                                lo = u * P - off
                                nc.tensor.matmul(
                                    avp[:, u, 0:D + 1],
                                    lhsT=pt[:, hi, lo:lo + P],
                                    rhs=v_aug[:, j, h, :],
                                    start=(u == 0 and idx == 0),
                                    stop=(idx == n - 1),
                                    skip_group_check=True)
                                rows += D + 1
                        rc = rcP.tile([P, 4, 1], f32, name="rc", tag="rc",
                                      bufs=4)
                        nc.vector.reciprocal(rc, avp[:, :, D:D + 1])
                        nc.vector.tensor_mul(
                            att_nat[:, 4 * c:4 * c + 4, h, :],
                            avp[:, :, 0:D],
                            rc.broadcast_to([P, 4, D]))
                        pe(rows)
                    return em

                return [head(0), head(1)]

            # ---- main interleaved emission ----
            # hp order (0,2,1,3): the first chunks use heads on quarters
            # 0..2 only, so nothing waits on the kq8b fix-up DMAs while the
            # input-load train still occupies the DMA engines.
            # c order (1,2,3,0): the epilogue chunk is the small c=0 one,
            # so little work trails the final exp
            c_order = list(range(qc_n))
            chunk_seq = [(hp, c) for c in c_order for hp in (0, 2, 1, 3)]
            done_hp = set()
            pending = None   # (hp, c, av emitter list)

            attTc_map = {}

            def queue_transposes(pc, cb):
                """Transposes of head pair cb for token chunk pc -> prio,
                as soon as that pair's AV drains."""
                if pc not in attTc_map:
                    attTc_map[pc] = atP.tile([P, VB, 4, P], bf16,
                                             name="attTc", tag="attTc",
                                             bufs=2)
                a = attTc_map[pc]
                for tt in range(4 * pc, 4 * pc + 4):
                    prio.append((lambda tt=tt, cb=cb, a=a:
                                 emit_transpose(tt, cb, a)))

            def queue_proj(pc, alt=False):
                a = attTc_map.pop(pc)
                for mb in range(KT):
                    tag = "avp" if (alt and mb % 2) else "qp"
                    prio.append((lambda mb=mb, pc=pc, a=a, tag=tag:
                                 emit_proj_unit(mb, pc, a, tag)))

            for hp, c in chunk_seq:
                force_kq(c, hp // 2)
                s_ems, pts = s_block_emitters(hp, c)
                avq = []
                if pending is not None:
                    php, pc, avq = pending
                    force_v(4 * pc + 3, php)
                for bi, em in enumerate(s_ems):
                    em()
                    if bi == 3:
                        # pull upcoming kq units (and their loads) in
                        # under this chunk's exp cover rather than at
                        # the point of need
                        if hp == 0:
                            force_kq(c, 1)
                        elif hp == 1 and c + 1 < qc_n:
                            force_kq(c + 1, 0)
                        elif hp == 3 and c + 1 < qc_n:
                            force_kq(c + 1, 1)
                    if bi >= 2 and avq:
                        avq.pop(0)()
                    emit_fills()
                while avq:
                    avq.pop(0)()
                    emit_fills()
                if pending is not None:
                    queue_transposes(pc, php)
                    done_hp.add((php, pc))
                    if all((h2, pc) in done_hp for h2 in range(4)):
                        queue_proj(pc)
                pending = (hp, c, av_emitters(hp, c, pts))

            php, pc, avq = pending
            force_v(4 * pc + 3, php)
            for em in avq:
                em()
                while prio:
                    prio.pop(0)()
            queue_transposes(pc, php)
            while prio:
                prio.pop(0)()
            queue_proj(pc, alt=True)
            while prio:
                prio.pop(0)()
            while fills:
                fn, key = fills.pop(0)
                fn()

    nc.compile()
    return nc


def _get_compiled(t=T):
    if t not in _compiled:
        _compiled[t] = _build(t)
    return _compiled[t]


def make_in_maps(x, W_qkv, W_proj):
    bf = ml_dtypes.bfloat16
    f8 = ml_dtypes.float8_e4m3
    x = np.asarray(x, dtype=np.float32)
    W_qkv = np.asarray(W_qkv, dtype=np.float32)
    W_proj = np.asarray(W_proj, dtype=np.float32)
    ident = np.eye(P, dtype=np.float32).astype(bf)
    in_maps = []
    for core in range(8):
        b, g = core // 2, core % 2
        xT = np.ascontiguousarray(x[b].T)           # [C, T]
        Wk = W_qkv[:, g * CG:(g + 1) * CG]
        Wq = W_qkv[:, C + g * CG:C + (g + 1) * CG]
        Wv = W_qkv[:, 2 * C + g * CG:2 * C + (g + 1) * CG]
        # permute k/q columns so the QKV matmul output lands directly in
        # the kq8a/kq8d layouts (see _build):
        #  per s-slot: cols [96a, 96a+96) = A-block a=2kq+e, slot-heads
        #  0..2; cols [384+64e, +64) = D-block (head 3's k then q)
        wkq8 = np.empty((C, C), dtype=np.float32)
        for kqi, W in ((0, Wk), (1, Wq)):
            Wr = (W * WS).reshape(C, 2, 4, 2, 32)      # [C, s, h', e, r]
            for s in range(2):
                for e in range(2):
                    a = 2 * kqi + e
                    lo = 384 * s + 96 * a
                    wkq8[:, lo:lo + 96] = Wr[:, s, 0:3, e, :].reshape(C, 96)
            for e in range(2):
                d = 2 * kqi + e
                lo = 768 + 64 * d
                wkq8[:, lo:lo + 32] = Wr[:, 0, 3, e, :]
                wkq8[:, lo + 32:lo + 64] = Wr[:, 1, 3, e, :]
        in_maps.append({
            "xT": xT.astype(bf),
            "x8T": xT.astype(f8),
            "wkq8": wkq8.astype(f8),
            "wv": np.ascontiguousarray(Wv).astype(bf),
            "wp": np.ascontiguousarray(
                W_proj[g * CG:(g + 1) * CG, :]).astype(bf),
            "ident": ident,
        })
    return in_maps


def _run_axon_nodonate(nc, in_maps, n_cores=8):
    """Execute via PJRT/shard_map WITHOUT output-buffer donation.

    bass2jax.run_bass_via_pjrt donates the zero output operands; under the
    axon transport that donation intermittently corrupts multi-core results.
    This kernel writes every element of its output, so donation is not
    needed for correctness -- pass non-donated zero operands instead.
    """
    import jax
    from jax.sharding import Mesh, PartitionSpec
    from jax.experimental.shard_map import shard_map
    import concourse.mybir as mybir
    from concourse.bass2jax import _bass_exec_p, install_neuronx_cc_hook

    install_neuronx_cc_hook()
    in_names, out_names, out_avals = [], [], []
    for alloc in nc.m.functions[0].allocations:
        if not isinstance(alloc, mybir.MemoryLocationSet):
            continue
        name = alloc.memorylocations[0].name
        if alloc.kind == "ExternalInput":
            in_names.append(name)
        elif alloc.kind == "ExternalOutput":
            out_names.append(name)
            out_avals.append(jax.core.ShapedArray(
                tuple(alloc.tensor_shape), mybir.dt.np(alloc.dtype)))
    n_params = len(in_names)
    all_names = in_names + out_names
    pid_name = nc.partition_id_tensor.name if nc.partition_id_tensor else None

    def _body(*args):
        return tuple(_bass_exec_p.bind(
            *args,
            out_avals=tuple(out_avals),
            in_names=tuple(all_names),
            out_names=tuple(out_names),
            lowering_input_output_aliases=(),
            sim_require_finite=True,
            sim_require_nnan=True,
            nc=nc,
        ))

    devices = jax.devices()[:n_cores]
    mesh = Mesh(np.asarray(devices), ("core",))
    fn = jax.jit(
        shard_map(_body, mesh=mesh,
                  in_specs=(PartitionSpec("core"),) * (n_params + len(out_names)),
                  out_specs=(PartitionSpec("core"),) * len(out_names),
                  check_rep=False),
        keep_unused=True)
    concat_in = [
        np.concatenate([
            np.asarray(in_maps[c].get(
                nm, np.array([[c]], dtype=np.uint32) if nm == pid_name
                else None))
            for c in range(n_cores)], 0)
        for nm in in_names
    ]
    concat_zeros = [
        np.zeros((n_cores * a.shape[0], *a.shape[1:]), a.dtype)
        for a in out_avals
    ]
    out = fn(*concat_in, *concat_zeros)
    return [
        {nm: np.asarray(out[i]).reshape(n_cores, *out_avals[i].shape)[c]
         for i, nm in enumerate(out_names)}
        for c in range(n_cores)
    ]


def kernel(x, W_qkv, W_proj, _trace=False):
    from concourse._compat import axon_active

    nc = _get_compiled()
    in_maps = make_in_maps(x, W_qkv, W_proj)
    if axon_active():
        results = _run_axon_nodonate(nc, in_maps)
    else:
        import concourse.bass_utils as bass_utils
        res = bass_utils.run_bass_kernel_spmd(
            nc, in_maps, core_ids=list(range(8)), trace=_trace)
        if _trace:
            kernel.last_results = res
        results = res.results
    y = np.zeros((B, T, C), np.float32)
    for core in range(8):
        y[core // 2] += results[core]["y"].T
    return y


# revision 67
# speedup vs baseline: 1.3933x; 1.0045x over previous
"""Causal self-attention Trainium2 kernel (fp8 DoubleRow + natural-AV).

Problem: y = CausalSelfAttention(x) with B=4, T=2048, C=1024, H=16 heads,
head_dim D=64, qkv split order (k, q, v), softmax scale C**-0.5.

Sharding (8 cores): core = 2*b + g  -> batch b in 0..3, head-group g in 0..1
(8 heads per group).  Each core computes, for its batch and its 8 heads:
qkv partial matmuls, causal attention, and the partial output projection
y_partial = att_out @ W_proj[rows of this head group].  The host sums the two
partial projections per batch.

Key speed tricks (tuned against the concourse instruction cost model):
  * k/q QKV matmuls run in fp8(e4m3) with perf_mode=DoubleRow: each
    instruction contracts TWO 128-deep k-tiles at 0.5 cycles/row.
    Weights are pre-scaled by WS=32 on the host so fp8 keeps precision;
    the scale is folded into the softmax exp scale (1/WS^2).
  * S = q^T k runs in fp8 DoubleRow too: D=64 is split into two 32-halves
    stored at different free offsets on quarter partition ranges
    (head h lives on partitions 32*(h%4)..+31).  The host permutes the
    W_qkv columns so the QKV matmul output lands directly in this layout
    (PSUM->SBUF copies stay partition-preserving).
  * AV runs in the natural [q, d] orientation: lhsT = P^T block (exp
    output), rhs = [V_h | ones] so N=65 streamed rows per k-tile instead
    of 512, and causal sparsity is exploited per 128-token q-tile.
    The ones column gives the softmax denominator; a per-partition
    reciprocal + broadcast multiply normalizes.  Four q-tile accumulation
    chains share one PSUM bank (only the first matmul in the bank sets
    start=True; the bank-wide pending-zero covers the other chains).
  * att comes out token-major; PE transpose (identity matmul) flips it to
    channel-major for the bf16 projection.
  * v and proj stay bf16: fp8 there would inject ~2-3% output error.
  * The attention stream is ACT(exp)-bound.  Emission interleaves, at
    S-block granularity: S blocks of chunk X+1, AV chains of chunk X, and
    qkv/v/transpose/proj filler units paced so that emitted PE rows track
    emitted ACT row-equivalents (keeps the PE p-state ramp hot and the
    ACT queue never empty).
"""

import numpy as np
import ml_dtypes

B, T, C, H = 4, 2048, 1024, 16
D = C // H          # 64
HPC = H // 2        # 8 heads per core
CG = C // 2         # 512 channels per head group
P = 128
KT = C // P         # 8 contraction tiles over C
TT = T // P         # 16 token tiles
QC = T // 512       # 4 q chunks of 512
VB = CG // P        # 4 att/channel blocks
WS = 32.0           # fp8 weight pre-scale for k/q

_compiled = {}


def _build(t=T):
    import concourse.bacc as bacc
    import concourse.tile as tile
    import concourse.mybir as mybir

    f32 = mybir.dt.float32
    bf16 = mybir.dt.bfloat16
    f8 = mybir.dt.float8e4
    Exp = mybir.ActivationFunctionType.Exp
    DR = mybir.MatmulPerfMode.DoubleRow

    tt_n = t // P
    qc_n = t // 512
    SCALE = float(C) ** -0.5 / (WS * WS)

    nc = bacc.Bacc("TRN2", target_bir_lowering=False, debug=False,
                   num_devices=8)

    xT_d = nc.dram_tensor("xT", [C, t], bf16, kind="ExternalInput")
    x8_d = nc.dram_tensor("x8T", [C, t], f8, kind="ExternalInput")
    wkq_d = nc.dram_tensor("wkq8", [C, C], f8, kind="ExternalInput")
    wv_d = nc.dram_tensor("wv", [C, CG], bf16, kind="ExternalInput")
    wp_d = nc.dram_tensor("wp", [CG, C], bf16, kind="ExternalInput")
    id_d = nc.dram_tensor("ident", [P, P], bf16, kind="ExternalInput")
    y_d = nc.dram_tensor("y", [C, t], f32, kind="ExternalOutput")

    with tile.TileContext(nc) as tc:
        with (
            tc.tile_pool(name="persist", bufs=1) as persist,
            tc.tile_pool(name="psA", bufs=2, space="PSUM") as psA,
            tc.tile_pool(name="ptP", bufs=18) as ptP,
            tc.tile_pool(name="ptdP", bufs=8) as ptdP,
            tc.tile_pool(name="rcP", bufs=4) as rcP,
            tc.tile_pool(name="atP", bufs=2) as atP,
        ):
            xT = persist.tile([P, KT, t], bf16)
            x8 = persist.tile([P, KT, t], f8)
            wkq_sb = persist.tile([P, KT, C], f8)
            wv_sb = persist.tile([P, KT, CG], bf16)
            wp_sb = persist.tile([P, VB, C], bf16)
            id_sb = persist.tile([P, P], bf16)
            # k/q activations in fp8, laid out so every S operand starts
            # at partition 0/32/64 (engine AP bases cannot encode 96):
            #  kq8a[32Q+r, kq, s, e, tok] = slot-head Q in {0,1,2}
            #  kq8d[32s+r, kq, e, tok]    = slot-head 3 of slot s
            kq8a = persist.tile([96, 2, 2, 2, t], f8)
            kq8d = persist.tile([64, 2, 2, t], f8)
            v_aug = persist.tile([P, tt_n, HPC, D + 1], bf16)
            att_nat = persist.tile([P, tt_n, HPC, D], bf16)
            ybuf = persist.tile([P, KT, 512], f32)

            # PE warm-up: dependency-free matmuls run during the input-DMA
            # window so the p-state ramp is hot when real work starts.
            wu_a = persist.tile([P, P], bf16)
            wu_b = persist.tile([P, 512], bf16)
            nc.vector.memset(wu_a, 0.0)
            nc.vector.memset(wu_b, 0.0)
            for _ in range(12):
                wps = psA.tile([P, 512], f32, name="wups", tag="qp", bufs=2)
                nc.tensor.matmul(wps, lhsT=wu_a, rhs=wu_b,
                                 start=True, stop=True,
                                 skip_group_check=True)

            # ---- input loads, ordered so token-chunk 0 lands first ----
            wkq_r = wkq_d.ap().rearrange("(kt p) m -> p kt m", p=P)
            x8_r = x8_d.ap().rearrange("(kt p) n -> p kt n", p=P)
            xT_r = xT_d.ap().rearrange("(kt p) n -> p kt n", p=P)
            wv_r = wv_d.ap().rearrange("(kt p) m -> p kt m", p=P)
            wp_r = wp_d.ap().rearrange("(kt p) m -> p kt m", p=P)
            nc.sync.dma_start(wkq_sb[:, :, 0:384], wkq_r[:, :, 0:384])
            nc.sync.dma_start(x8[:, :, 0:512], x8_r[:, :, 0:512])
            nc.sync.dma_start(wkq_sb[:, :, 384:C], wkq_r[:, :, 384:C])
            nc.sync.dma_start(wv_sb[:, :, 0:2 * D], wv_r[:, :, 0:2 * D])
            nc.sync.dma_start(xT[:, :, 0:512], xT_r[:, :, 0:512])
            nc.sync.dma_start(id_sb, id_d.ap())

            def emit_wv_load(hp):
                lo = 2 * hp * D
                nc.sync.dma_start(wv_sb[:, :, lo:lo + 2 * D],
                                  wv_r[:, :, lo:lo + 2 * D])
            # only the ones-column needs initializing; v units fill 0:D
            nc.vector.memset(v_aug[:, :, :, D:D + 1], 1.0)

            def emit_load(c):
                """Stream token chunk c of x8/xT (and wp alongside c=1);
                lazy so the DMA engines stay available for the kq8b
                fix-up transfers during the early attention chunks."""
                lo, hi = c * 512, (c + 1) * 512
                nc.sync.dma_start(x8[:, :, lo:hi], x8_r[:, :, lo:hi])
                nc.sync.dma_start(xT[:, :, lo:hi], xT_r[:, :, lo:hi])
                if c == 2:
                    nc.sync.dma_start(wp_sb, wp_r)

            # ---- pacing state ----
            st8 = {"pe": 0, "act": 0}
            import kernel as _km
            _km._dbg_map = {}

            def _dbg(inst, label):
                try:
                    _km._dbg_map[inst.inst.name] = label
                except Exception:
                    try:
                        _km._dbg_map[inst.name] = label
                    except Exception:
                        pass

            def pe(rows):
                st8["pe"] += rows

            def act(rows):
                st8["act"] += rows

            # ---- unit emitters ----
            # wkq column layout (1024 cols):
            #   [384s + 96a, +96): A-block (s, a=2kq+e), slot-heads 0..2
            #   [768 + 64d, +64):  D-block d=2kq+e: head 3 of s0 (32),
            #                      then head 3 of s1 (32)
            def emit_kq_unit(kq, s, e, c):
                """One A-block (M=96, slot-heads 0..2) of k or q for one
                512-tok chunk, fp8 DoubleRow over kt pairs."""
                a = 2 * kq + e
                lo = 384 * s + 96 * a
                ps = psA.tile([96, 512], f32, name="ps", tag="qp", bufs=2)
                for kt in range(4):
                    nc.tensor.matmul(
                        ps,
                        lhsT=wkq_sb[:, 2 * kt:2 * kt + 2, lo:lo + 96],
                        rhs=x8[:, 2 * kt:2 * kt + 2, c * 512:(c + 1) * 512],
                        start=(kt == 0), stop=(kt == 3),
                        perf_mode=DR, skip_group_check=True)
                nc.vector.tensor_copy(
                    kq8a[:, kq, s, e, c * 512:(c + 1) * 512], ps)
                pe(1024)

            def emit_kqd_unit(kq, e, c):
                """One D-block (M=64, both slots' head 3) for one
                512-tok chunk."""
                d = 2 * kq + e
                lo = 768 + 64 * d
                ps = psA.tile([64, 512], f32, name="psd", tag="qp", bufs=2)
                for kt in range(4):
                    nc.tensor.matmul(
                        ps,
                        lhsT=wkq_sb[:, 2 * kt:2 * kt + 2, lo:lo + 64],
                        rhs=x8[:, 2 * kt:2 * kt + 2, c * 512:(c + 1) * 512],
                        start=(kt == 0), stop=(kt == 3),
                        perf_mode=DR, skip_group_check=True)
                nc.vector.tensor_copy(
                    kq8d[:, kq, e, c * 512:(c + 1) * 512], ps)
                pe(1024)

            def emit_v_unit(tt, hp):
                """v for one 128-token tile and one head pair (natural
                layout, bf16).  Head-pair granularity keeps the forced
                emission ahead of AV small enough to hide under the
                exp stream."""
                ps = psA.tile([P, 2 * D], f32, name="psv", tag="qp", bufs=2)
                for kt in range(KT):
                    nc.tensor.matmul(
                        ps,
                        lhsT=xT[:, kt, tt * P:(tt + 1) * P],
                        rhs=wv_sb[:, kt, 2 * hp * D:(2 * hp + 2) * D],
                        start=(kt == 0), stop=(kt == KT - 1),
                        skip_group_check=True)
                nc.vector.tensor_copy(
                    v_aug[:, tt, 2 * hp:2 * hp + 2, 0:D],
                    ps.rearrange("p (h d) -> p h d", d=D))
                pe(1024)

            def emit_transpose(tt, cb, attTc):
                """att_nat[:, tt, heads 2cb..2cb+1] -> attTc channel-major."""
                pst = psA.tile([P, P], bf16, name="pst", tag="qp", bufs=2)
                nc.tensor.transpose(
                    pst, att_nat[:, tt, 2 * cb:2 * cb + 2, :], id_sb)
                nc.vector.tensor_copy(attTc[:, cb, tt % 4, :], pst)
                pe(128)

            def emit_proj_unit(mb, c, attTc, tag="qp"):
                ps = psA.tile([P, 512], f32, name="psp", tag=tag, bufs=2)
                for kt in range(VB):
                    nc.tensor.matmul(
                        ps,
                        lhsT=wp_sb[:, kt, mb * P:(mb + 1) * P],
                        rhs=attTc[:, kt, :, :],
                        start=(kt == 0), stop=(kt == VB - 1),
                        skip_group_check=True)
                nc.vector.tensor_copy(ybuf[:, mb, :], ps)
                nc.sync.dma_start(
                    y_d[mb * P:(mb + 1) * P, c * 512:(c + 1) * 512],
                    ybuf[:, mb, :])
                pe(2048)

            # ---- filler queues ----
            # prio: transposes/proj (drain ASAP, ahead of pacing)
            # fills: kq/v units, paced against the ACT stream
            prio = []
            fills = []  # (fn, key); key = ("kq", c, s) / ("v"|"ld", i, _)
            for hp in (2, 1, 3):
                fills.append(((lambda hp=hp: emit_wv_load(hp)),
                              ("lw", hp, 0)))
            for c in range(qc_n):
                if c > 0:
                    fills.append(((lambda c=c: emit_load(c)), ("ld", c, 0)))
                for s in (0, 1):
                    for kq in (0, 1):
                        for e in (0, 1):
                            fills.append(
                                ((lambda kq=kq, s=s, e=e, c=c:
                                  emit_kq_unit(kq, s, e, c)),
                                 ("kq", c, s)))
                for kq in (0, 1):
                    for e in (0, 1):
                        fills.append(
                            ((lambda kq=kq, e=e, c=c:
                              emit_kqd_unit(kq, e, c)), ("kq", c, 0)))
                for hp in (0, 2, 1, 3):
                    for tt in range(4 * c, 4 * c + 4):
                        fills.append(
                            ((lambda tt=tt, hp=hp: emit_v_unit(tt, hp)),
                             ("v", tt, hp)))

            LEAD = 6000  # rows of PE work kept queued ahead of ACT
            st8["dummies"] = 0

            soon = []

            def pull_soon(match):
                """Move matching fills into the spread-out soon queue."""
                i = 0
                while i < len(fills):
                    fn, key = fills[i]
                    if match(key):
                        soon.append(fills.pop(i))
                    else:
                        i += 1

            def emit_fills():
                """Priority units first (up to 2 per call), then one unit
                from the look-ahead queue, then pacing fillers.  When
                everything is drained but the PE is behind the ACT stream,
                emit keep-warm matmuls so the p-state ramp stays hot."""
                n = 0
                while prio and n < 2:
                    prio.pop(0)()
                    n += 1
                if soon and st8["pe"] < st8["act"] + 2 * LEAD:
                    fn, key = soon.pop(0)
                    fn()
                elif fills and st8["pe"] < st8["act"] + LEAD:
                    fn, key = fills.pop(0)
                    fn()
                while (not fills and not prio and st8["dummies"] < 200
                       and st8["pe"] < st8["act"]):
                    wps = psA.tile([P, 512], f32, name="wrm", tag="qp",
                                   bufs=2)
                    nc.tensor.matmul(wps, lhsT=wu_a, rhs=wu_b,
                                     start=True, stop=True,
                                     skip_group_check=True)
                    pe(512)
                    st8["dummies"] += 1

            def _force(q, cond):
                i = 0
                while i < len(q):
                    fn, key = q[i]
                    if cond(key):
                        q.pop(i)
                        fn()
                    else:
                        i += 1

            def force_ld(c):
                for q in (soon, fills):
                    _force(q, lambda k: k[0] == "ld" and k[1] <= c)

            def force_kq(c, s):
                force_ld(c)
                for q in (soon, fills):
                    _force(q, lambda k: k[0] == "kq" and k[1] <= c
                           and k[2] in (s, 0) if k[0] == "kq" else False)

            def force_v(tt, hp):
                force_ld(tt // 4)
                for q in (soon, fills):
                    _force(q, lambda k:
                           (k[0] == "v" and k[1] <= tt and k[2] == hp)
                           or (k[0] == "lw" and k[1] == hp))

            # ---- attention building blocks ----
            def s_block_emitters(hp, c):
                """Per-block closures for S^T + exp of head pair hp,
                chunk c; returns (emitters, pts) where pts is filled in
                as blocks run."""
                s_h = hp // 2
                q0, q1 = (2 * hp) % 4, (2 * hp + 1) % 4
                nfull = 4 * c
                pts = []
                ems = []

                def kq_op(kq, Q, c0, c1):
                    if Q == 3:
                        lo = 32 * s_h
                        return kq8d[lo:lo + 32, kq, :, c0:c1]
                    return kq8a[32 * Q:32 * Q + 32, kq, s_h, :, c0:c1]

                def full_block(j):
                    def em():
                        st = psA.tile([P, 2, 512], f32, name="st", tag="st",
                                      bufs=2)
                        for hi, Q in ((0, q0), (1, q1)):
                            nc.tensor.matmul(
                                st[:, hi, :],
                                lhsT=kq_op(0, Q, j * P, (j + 1) * P),
                                rhs=kq_op(1, Q, c * 512, (c + 1) * 512),
                                start=True, stop=True,
                                perf_mode=DR, skip_group_check=True)
                        pt = ptP.tile([P, 2, 512], bf16, name="pt",
                                      tag="pt")
                        _i = nc.scalar.activation(pt, st, Exp, scale=SCALE)
                        _dbg(_i, f"exp S({hp},{c})j{j}")
                        pts.append((pt, j, 0))
                        pe(512)
                        act(2 * 1024 + 444)
                    return em

                def diag_block(dj):
                    def em():
                        j = nfull + dj
                        off = P * dj
                        w = 512 - off
                        st = psA.tile([P, 2, 512], f32, name="std",
                                      tag="st")
                        for hi, Q in ((0, q0), (1, q1)):
                            nc.tensor.matmul(
                                st[:, hi, 0:w],
                                lhsT=kq_op(0, Q, j * P, (j + 1) * P),
                                rhs=kq_op(1, Q, c * 512 + off,
                                          (c + 1) * 512),
                                start=True, stop=True,
                                perf_mode=DR, skip_group_check=True)
                        pt = ptdP.tile([P, 2, 512], bf16, name="ptd",
                                       tag="ptd")
                        _i = nc.scalar.activation(pt[:, :, 0:w],
                                                  st[:, :, 0:w],
                                                  Exp, scale=SCALE)
                        _dbg(_i, f"exp S({hp},{c})d{dj}")
                        nc.gpsimd.affine_select(
                            pt[:, :, 0:P], pt[:, :, 0:P],
                            pattern=[[0, 2], [1, P]],
                            compare_op=mybir.AluOpType.is_ge,
                            fill=0.0, base=0, channel_multiplier=-1)
                        pts.append((pt, j, off))
                        pe(w)
                        act(4 * w + 444)
                    return em

                for j in range(nfull):
                    ems.append(full_block(j))
                for dj in range(4):
                    ems.append(diag_block(dj))
                return ems, pts

            def av_emitters(hp, c, pts):
                """Per-head closures: 4 accumulation chains + drain."""
                ems = []

                def head(hi):
                    def em():
                        h = 2 * hp + hi
                        avp = psA.tile([P, 4, P], f32, name=f"avp{hi}",
                                       tag="avp", bufs=2)
                        rows = 0
                        for u in range(4):
                            tq = 4 * c + u
                            chain = [pj for pj in pts if pj[1] <= tq]
                            n = len(chain)
                            for idx, (pt, j, off) in enumerate(chain):
                                lo = u * P - off
                                nc.tensor.matmul(
                                    avp[:, u, 0:D + 1],
                                    lhsT=pt[:, hi, lo:lo + P],
                                    rhs=v_aug[:, j, h, :],
                                    start=(u == 0 and idx == 0),
                                    stop=(idx == n - 1),
                                    skip_group_check=True)
                                rows += D + 1
                        rc = rcP.tile([P, 4, 1], f32, name="rc", tag="rc",
                                      bufs=4)
                        nc.vector.reciprocal(rc, avp[:, :, D:D + 1])
                        nc.vector.tensor_mul(
                            att_nat[:, 4 * c:4 * c + 4, h, :],
                            avp[:, :, 0:D],
                            rc.broadcast_to([P, 4, D]))
                        pe(rows)
                    return em

                return [head(0), head(1)]

            # ---- main interleaved emission ----
            # hp order (0,2,1,3): the first chunks use heads on quarters
            # 0..2 only, so nothing waits on the kq8b fix-up DMAs while the
            # input-load train still occupies the DMA engines.
            # c order (1,2,3,0): the epilogue chunk is the small c=0 one,
            # so little work trails the final exp
            c_order = list(range(qc_n))
            chunk_seq = [(hp, c) for c in c_order for hp in (0, 2, 1, 3)]
            done_hp = set()
            pending = None   # (hp, c, av emitter list)

            attTc_map = {}

            def queue_transposes(pc, cb):
                """Transposes of head pair cb for token chunk pc -> prio,
                as soon as that pair's AV drains."""
                if pc not in attTc_map:
                    attTc_map[pc] = atP.tile([P, VB, 4, P], bf16,
                                             name="attTc", tag="attTc",
                                             bufs=2)
                a = attTc_map[pc]
                for tt in range(4 * pc, 4 * pc + 4):
                    prio.append((lambda tt=tt, cb=cb, a=a:
                                 emit_transpose(tt, cb, a)))

            def queue_proj(pc, alt=False):
                a = attTc_map.pop(pc)
                for mb in range(KT):
                    tag = "avp" if (alt and mb % 2) else "qp"
                    prio.append((lambda mb=mb, pc=pc, a=a, tag=tag:
                                 emit_proj_unit(mb, pc, a, tag)))

            for ci, (hp, c) in enumerate(chunk_seq):
                force_kq(c, hp // 2)
                s_ems, pts = s_block_emitters(hp, c)
                avq = []
                if pending is not None:
                    php, pc, avq = pending
                    force_v(4 * pc + 3, php)
                for bi, em in enumerate(s_ems):
                    em()
                    if bi == 3:
                        # pull upcoming kq units (and their loads) in
                        # under this chunk's exp cover rather than at
                        # the point of need
                        if hp == 0:
                            force_kq(c, 1)
                            force_ld(min(c + 1, qc_n - 1))
                        elif hp == 1 and c + 1 < qc_n:
                            force_kq(c + 1, 0)
                        elif hp == 3 and c + 1 < qc_n:
                            force_kq(c + 1, 1)
                    if bi >= 2 and avq:
                        avq.pop(0)()
                    emit_fills()
                while avq:
                    avq.pop(0)()
                    emit_fills()
                if pending is not None:
                    queue_transposes(pc, php)
                    done_hp.add((php, pc))
                    if all((h2, pc) in done_hp for h2 in range(4)):
                        queue_proj(pc)
                pending = (hp, c, av_emitters(hp, c, pts))

            php, pc, avq = pending
            force_v(4 * pc + 3, php)
            for em in avq:
                em()
                while prio:
                    prio.pop(0)()
            queue_transposes(pc, php)
            while prio:
                prio.pop(0)()
            queue_proj(pc, alt=True)
            while prio:
                prio.pop(0)()
            while fills:
                fn, key = fills.pop(0)
                fn()

    nc.compile()
    return nc


def _get_compiled(t=T):
    if t not in _compiled:
        _compiled[t] = _build(t)
    return _compiled[t]


def make_in_maps(x, W_qkv, W_proj):
    bf = ml_dtypes.bfloat16
    f8 = ml_dtypes.float8_e4m3
    x = np.asarray(x, dtype=np.float32)
    W_qkv = np.asarray(W_qkv, dtype=np.float32)
    W_proj = np.asarray(W_proj, dtype=np.float32)
    ident = np.eye(P, dtype=np.float32).astype(bf)
    in_maps = []
    for core in range(8):
        b, g = core // 2, core % 2
        xT = np.ascontiguousarray(x[b].T)           # [C, T]
        Wk = W_qkv[:, g * CG:(g + 1) * CG]
        Wq = W_qkv[:, C + g * CG:C + (g + 1) * CG]
        Wv = W_qkv[:, 2 * C + g * CG:2 * C + (g + 1) * CG]
        # permute k/q columns so the QKV matmul output lands directly in
        # the kq8a/kq8d layouts (see _build):
        #  per s-slot: cols [96a, 96a+96) = A-block a=2kq+e, slot-heads
        #  0..2; cols [384+64e, +64) = D-block (head 3's k then q)
        wkq8 = np.empty((C, C), dtype=np.float32)
        for kqi, W in ((0, Wk), (1, Wq)):
            Wr = (W * WS).reshape(C, 2, 4, 2, 32)      # [C, s, h', e, r]
            for s in range(2):
                for e in range(2):
                    a = 2 * kqi + e
                    lo = 384 * s + 96 * a
                    wkq8[:, lo:lo + 96] = Wr[:, s, 0:3, e, :].reshape(C, 96)
            for e in range(2):
                d = 2 * kqi + e
                lo = 768 + 64 * d
                wkq8[:, lo:lo + 32] = Wr[:, 0, 3, e, :]
                wkq8[:, lo + 32:lo + 64] = Wr[:, 1, 3, e, :]
        in_maps.append({
            "xT": xT.astype(bf),
            "x8T": xT.astype(f8),
            "wkq8": wkq8.astype(f8),
            "wv": np.ascontiguousarray(Wv).astype(bf),
            "wp": np.ascontiguousarray(
                W_proj[g * CG:(g + 1) * CG, :]).astype(bf),
            "ident": ident,
        })
    return in_maps


def _run_axon_nodonate(nc, in_maps, n_cores=8):
    """Execute via PJRT/shard_map WITHOUT output-buffer donation.

    bass2jax.run_bass_via_pjrt donates the zero output operands; under the
    axon transport that donation intermittently corrupts multi-core results.
    This kernel writes every element of its output, so donation is not
    needed for correctness -- pass non-donated zero operands instead.
    """
    import jax
    from jax.sharding import Mesh, PartitionSpec
    from jax.experimental.shard_map import shard_map
    import concourse.mybir as mybir
    from concourse.bass2jax import _bass_exec_p, install_neuronx_cc_hook

    install_neuronx_cc_hook()
    in_names, out_names, out_avals = [], [], []
    for alloc in nc.m.functions[0].allocations:
        if not isinstance(alloc, mybir.MemoryLocationSet):
            continue
        name = alloc.memorylocations[0].name
        if alloc.kind == "ExternalInput":
            in_names.append(name)
        elif alloc.kind == "ExternalOutput":
            out_names.append(name)
            out_avals.append(jax.core.ShapedArray(
                tuple(alloc.tensor_shape), mybir.dt.np(alloc.dtype)))
    n_params = len(in_names)
    all_names = in_names + out_names
    pid_name = nc.partition_id_tensor.name if nc.partition_id_tensor else None

    def _body(*args):
        return tuple(_bass_exec_p.bind(
            *args,
            out_avals=tuple(out_avals),
            in_names=tuple(all_names),
            out_names=tuple(out_names),
            lowering_input_output_aliases=(),
            sim_require_finite=True,
            sim_require_nnan=True,
            nc=nc,
        ))

    devices = jax.devices()[:n_cores]
    mesh = Mesh(np.asarray(devices), ("core",))
    fn = jax.jit(
        shard_map(_body, mesh=mesh,
                  in_specs=(PartitionSpec("core"),) * (n_params + len(out_names)),
                  out_specs=(PartitionSpec("core"),) * len(out_names),
                  check_rep=False),
        keep_unused=True)
    concat_in = [
        np.concatenate([
            np.asarray(in_maps[c].get(
                nm, np.array([[c]], dtype=np.uint32) if nm == pid_name
                else None))
            for c in range(n_cores)], 0)
        for nm in in_names
    ]
    concat_zeros = [
        np.zeros((n_cores * a.shape[0], *a.shape[1:]), a.dtype)
        for a in out_avals
    ]
    out = fn(*concat_in, *concat_zeros)
    return [
        {nm: np.asarray(out[i]).reshape(n_cores, *out_avals[i].shape)[c]
         for i, nm in enumerate(out_names)}
        for c in range(n_cores)
    ]


def kernel(x, W_qkv, W_proj, _trace=False):
    from concourse._compat import axon_active

    nc = _get_compiled()
    in_maps = make_in_maps(x, W_qkv, W_proj)
    if axon_active():
        results = _run_axon_nodonate(nc, in_maps)
    else:
        import concourse.bass_utils as bass_utils
        res = bass_utils.run_bass_kernel_spmd(
            nc, in_maps, core_ids=list(range(8)), trace=_trace)
        if _trace:
            kernel.last_results = res
        results = res.results
    y = np.zeros((B, T, C), np.float32)
    for core in range(8):
        y[core // 2] += results[core]["y"].T
    return y


# revision 71
# speedup vs baseline: 1.3977x; 1.0032x over previous
"""Causal self-attention Trainium2 kernel (fp8 DoubleRow + natural-AV).

Problem: y = CausalSelfAttention(x) with B=4, T=2048, C=1024, H=16 heads,
head_dim D=64, qkv split order (k, q, v), softmax scale C**-0.5.

Sharding (8 cores): core = 2*b + g  -> batch b in 0..3, head-group g in 0..1
(8 heads per group).  Each core computes, for its batch and its 8 heads:
qkv partial matmuls, causal attention, and the partial output projection
y_partial = att_out @ W_proj[rows of this head group].  The host sums the two
partial projections per batch.

Key speed tricks (tuned against the concourse instruction cost model):
  * k/q QKV matmuls run in fp8(e4m3) with perf_mode=DoubleRow: each
    instruction contracts TWO 128-deep k-tiles at 0.5 cycles/row.
    Weights are pre-scaled by WS=32 on the host so fp8 keeps precision;
    the scale is folded into the softmax exp scale (1/WS^2).
  * S = q^T k runs in fp8 DoubleRow too: D=64 is split into two 32-halves
    stored at different free offsets on quarter partition ranges
    (head h lives on partitions 32*(h%4)..+31).  The host permutes the
    W_qkv columns so the QKV matmul output lands directly in this layout
    (PSUM->SBUF copies stay partition-preserving).
  * AV runs in the natural [q, d] orientation: lhsT = P^T block (exp
    output), rhs = [V_h | ones] so N=65 streamed rows per k-tile instead
    of 512, and causal sparsity is exploited per 128-token q-tile.
    The ones column gives the softmax denominator; a per-partition
    reciprocal + broadcast multiply normalizes.  Four q-tile accumulation
    chains share one PSUM bank (only the first matmul in the bank sets
    start=True; the bank-wide pending-zero covers the other chains).
  * att comes out token-major; PE transpose (identity matmul) flips it to
    channel-major for the bf16 projection.
  * v and proj stay bf16: fp8 there would inject ~2-3% output error.
  * The attention stream is ACT(exp)-bound.  Emission interleaves, at
    S-block granularity: S blocks of chunk X+1, AV chains of chunk X, and
    qkv/v/transpose/proj filler units paced so that emitted PE rows track
    emitted ACT row-equivalents (keeps the PE p-state ramp hot and the
    ACT queue never empty).
"""

import numpy as np
import ml_dtypes

B, T, C, H = 4, 2048, 1024, 16
D = C // H          # 64
HPC = H // 2        # 8 heads per core
CG = C // 2         # 512 channels per head group
P = 128
KT = C // P         # 8 contraction tiles over C
TT = T // P         # 16 token tiles
QC = T // 512       # 4 q chunks of 512
VB = CG // P        # 4 att/channel blocks
WS = 32.0           # fp8 weight pre-scale for k/q

_compiled = {}


def _build(t=T):
    import concourse.bacc as bacc
    import concourse.tile as tile
    import concourse.mybir as mybir

    f32 = mybir.dt.float32
    bf16 = mybir.dt.bfloat16
    f8 = mybir.dt.float8e4
    Exp = mybir.ActivationFunctionType.Exp
    DR = mybir.MatmulPerfMode.DoubleRow

    tt_n = t // P
    qc_n = t // 512
    SCALE = float(C) ** -0.5 / (WS * WS)

    nc = bacc.Bacc("TRN2", target_bir_lowering=False, debug=False,
                   num_devices=8)

    xT_d = nc.dram_tensor("xT", [C, t], bf16, kind="ExternalInput")
    x8_d = nc.dram_tensor("x8T", [C, t], f8, kind="ExternalInput")
    wkq_d = nc.dram_tensor("wkq8", [C, C], f8, kind="ExternalInput")
    wv_d = nc.dram_tensor("wv", [C, CG], bf16, kind="ExternalInput")
    wp_d = nc.dram_tensor("wp", [CG, C], bf16, kind="ExternalInput")
    id_d = nc.dram_tensor("ident", [P, P], bf16, kind="ExternalInput")
    y_d = nc.dram_tensor("y", [C, t], f32, kind="ExternalOutput")

    with tile.TileContext(nc) as tc:
        with (
            tc.tile_pool(name="persist", bufs=1) as persist,
            tc.tile_pool(name="psA", bufs=2, space="PSUM") as psA,
            tc.tile_pool(name="ptP", bufs=18) as ptP,
            tc.tile_pool(name="ptdP", bufs=8) as ptdP,
            tc.tile_pool(name="rcP", bufs=4) as rcP,
            tc.tile_pool(name="atP", bufs=2) as atP,
        ):
            xT = persist.tile([P, KT, t], bf16)
            x8 = persist.tile([P, KT, t], f8)
            wkq_sb = persist.tile([P, KT, C], f8)
            wv_sb = persist.tile([P, KT, CG], bf16)
            wp_sb = persist.tile([P, VB, C], bf16)
            id_sb = persist.tile([P, P], bf16)
            # k/q activations in fp8, laid out so every S operand starts
            # at partition 0/32/64 (engine AP bases cannot encode 96):
            #  kq8a[32Q+r, kq, s, e, tok] = slot-head Q in {0,1,2}
            #  kq8d[32s+r, kq, e, tok]    = slot-head 3 of slot s
            kq8a = persist.tile([96, 2, 2, 2, t], f8)
            kq8d = persist.tile([64, 2, 2, t], f8)
            v_aug = persist.tile([P, tt_n, HPC, D + 1], bf16)
            att_nat = persist.tile([P, tt_n, HPC, D], bf16)
            ybuf = persist.tile([P, KT, 512], f32)

            # PE warm-up: dependency-free matmuls run during the input-DMA
            # window so the p-state ramp is hot when real work starts.
            wu_a = persist.tile([P, P], bf16)
            wu_b = persist.tile([P, 512], bf16)
            nc.vector.memset(wu_a, 0.0)
            nc.vector.memset(wu_b, 0.0)
            for _ in range(12):
                wps = psA.tile([P, 512], f32, name="wups", tag="qp", bufs=2)
                nc.tensor.matmul(wps, lhsT=wu_a, rhs=wu_b,
                                 start=True, stop=True,
                                 skip_group_check=True)

            # ---- input loads, ordered so token-chunk 0 lands first ----
            wkq_r = wkq_d.ap().rearrange("(kt p) m -> p kt m", p=P)
            x8_r = x8_d.ap().rearrange("(kt p) n -> p kt n", p=P)
            xT_r = xT_d.ap().rearrange("(kt p) n -> p kt n", p=P)
            wv_r = wv_d.ap().rearrange("(kt p) m -> p kt m", p=P)
            wp_r = wp_d.ap().rearrange("(kt p) m -> p kt m", p=P)
            nc.sync.dma_start(wkq_sb[:, :, 0:384], wkq_r[:, :, 0:384])
            nc.sync.dma_start(x8[:, :, 0:512], x8_r[:, :, 0:512])
            nc.sync.dma_start(wkq_sb[:, :, 384:C], wkq_r[:, :, 384:C])
            nc.sync.dma_start(wv_sb[:, :, 0:2 * D], wv_r[:, :, 0:2 * D])
            nc.sync.dma_start(xT[:, :, 0:512], xT_r[:, :, 0:512])
            nc.sync.dma_start(id_sb, id_d.ap())

            def emit_wv_load(hp):
                lo = 2 * hp * D
                nc.sync.dma_start(wv_sb[:, :, lo:lo + 2 * D],
                                  wv_r[:, :, lo:lo + 2 * D])
            # only the ones-column needs initializing; v units fill 0:D
            nc.vector.memset(v_aug[:, :, :, D:D + 1], 1.0)

            def emit_load(c):
                """Stream token chunk c of x8/xT (and wp alongside c=1);
                lazy so the DMA engines stay available for the kq8b
                fix-up transfers during the early attention chunks."""
                lo, hi = c * 512, (c + 1) * 512
                nc.sync.dma_start(x8[:, :, lo:hi], x8_r[:, :, lo:hi])
                nc.sync.dma_start(xT[:, :, lo:hi], xT_r[:, :, lo:hi])
                if c == 2:
                    nc.sync.dma_start(wp_sb, wp_r)

            # ---- pacing state ----
            st8 = {"pe": 0, "act": 0}

            def pe(rows):
                st8["pe"] += rows

            def act(rows):
                st8["act"] += rows

            # ---- unit emitters ----
            # wkq column layout (1024 cols):
            #   [384s + 96a, +96): A-block (s, a=2kq+e), slot-heads 0..2
            #   [768 + 64d, +64):  D-block d=2kq+e: head 3 of s0 (32),
            #                      then head 3 of s1 (32)
            def emit_kq_unit(kq, s, e, c):
                """One A-block (M=96, slot-heads 0..2) of k or q for one
                512-tok chunk, fp8 DoubleRow over kt pairs."""
                a = 2 * kq + e
                lo = 384 * s + 96 * a
                ps = psA.tile([96, 512], f32, name="ps", tag="qp", bufs=2)
                for kt in range(4):
                    nc.tensor.matmul(
                        ps,
                        lhsT=wkq_sb[:, 2 * kt:2 * kt + 2, lo:lo + 96],
                        rhs=x8[:, 2 * kt:2 * kt + 2, c * 512:(c + 1) * 512],
                        start=(kt == 0), stop=(kt == 3),
                        perf_mode=DR, skip_group_check=True)
                nc.vector.tensor_copy(
                    kq8a[:, kq, s, e, c * 512:(c + 1) * 512], ps)
                pe(1024)

            def emit_kqd_unit(kq, e, c):
                """One D-block (M=64, both slots' head 3) for one
                512-tok chunk."""
                d = 2 * kq + e
                lo = 768 + 64 * d
                ps = psA.tile([64, 512], f32, name="psd", tag="qp", bufs=2)
                for kt in range(4):
                    nc.tensor.matmul(
                        ps,
                        lhsT=wkq_sb[:, 2 * kt:2 * kt + 2, lo:lo + 64],
                        rhs=x8[:, 2 * kt:2 * kt + 2, c * 512:(c + 1) * 512],
                        start=(kt == 0), stop=(kt == 3),
                        perf_mode=DR, skip_group_check=True)
                nc.vector.tensor_copy(
                    kq8d[:, kq, e, c * 512:(c + 1) * 512], ps)
                pe(1024)

            def emit_v_unit(tt, hp):
                """v for one 128-token tile and one head pair (natural
                layout, bf16).  Head-pair granularity keeps the forced
                emission ahead of AV small enough to hide under the
                exp stream."""
                ps = psA.tile([P, 2 * D], f32, name="psv", tag="qp", bufs=2)
                for kt in range(KT):
                    nc.tensor.matmul(
                        ps,
                        lhsT=xT[:, kt, tt * P:(tt + 1) * P],
                        rhs=wv_sb[:, kt, 2 * hp * D:(2 * hp + 2) * D],
                        start=(kt == 0), stop=(kt == KT - 1),
                        skip_group_check=True)
                nc.vector.tensor_copy(
                    v_aug[:, tt, 2 * hp:2 * hp + 2, 0:D],
                    ps.rearrange("p (h d) -> p h d", d=D))
                pe(1024)

            def emit_transpose(tt, cb, attTc):
                """att_nat[:, tt, heads 2cb..2cb+1] -> attTc channel-major."""
                pst = psA.tile([P, P], bf16, name="pst", tag="qp", bufs=2)
                nc.tensor.transpose(
                    pst, att_nat[:, tt, 2 * cb:2 * cb + 2, :], id_sb)
                nc.vector.tensor_copy(attTc[:, cb, tt % 4, :], pst)
                pe(128)

            def emit_proj_unit(mb, c, attTc, tag="qp"):
                ps = psA.tile([P, 512], f32, name="psp", tag=tag, bufs=2)
                for kt in range(VB):
                    nc.tensor.matmul(
                        ps,
                        lhsT=wp_sb[:, kt, mb * P:(mb + 1) * P],
                        rhs=attTc[:, kt, :, :],
                        start=(kt == 0), stop=(kt == VB - 1),
                        skip_group_check=True)
                nc.vector.tensor_copy(ybuf[:, mb, :], ps)
                nc.sync.dma_start(
                    y_d[mb * P:(mb + 1) * P, c * 512:(c + 1) * 512],
                    ybuf[:, mb, :])
                pe(2048)

            # ---- filler queues ----
            # prio: transposes/proj (drain ASAP, ahead of pacing)
            # fills: kq/v units, paced against the ACT stream
            prio = []
            fills = []  # (fn, key); key = ("kq", c, s) / ("v"|"ld", i, _)
            for hp in (2, 1, 3):
                fills.append(((lambda hp=hp: emit_wv_load(hp)),
                              ("lw", hp, 0)))
            for c in range(qc_n):
                if c > 0:
                    fills.append(((lambda c=c: emit_load(c)), ("ld", c, 0)))
                for s in (0, 1):
                    for kq in (0, 1):
                        for e in (0, 1):
                            fills.append(
                                ((lambda kq=kq, s=s, e=e, c=c:
                                  emit_kq_unit(kq, s, e, c)),
                                 ("kq", c, s)))
                for kq in (0, 1):
                    for e in (0, 1):
                        fills.append(
                            ((lambda kq=kq, e=e, c=c:
                              emit_kqd_unit(kq, e, c)), ("kq", c, 0)))
                for hp in (0, 2, 1, 3):
                    for tt in range(4 * c, 4 * c + 4):
                        fills.append(
                            ((lambda tt=tt, hp=hp: emit_v_unit(tt, hp)),
                             ("v", tt, hp)))

            LEAD = 8000  # rows of PE work kept queued ahead of ACT
            st8["dummies"] = 0

            soon = []

            def pull_soon(match):
                """Move matching fills into the spread-out soon queue."""
                i = 0
                while i < len(fills):
                    fn, key = fills[i]
                    if match(key):
                        soon.append(fills.pop(i))
                    else:
                        i += 1

            def emit_fills():
                """Priority units first (up to 2 per call), then one unit
                from the look-ahead queue, then pacing fillers.  When
                everything is drained but the PE is behind the ACT stream,
                emit keep-warm matmuls so the p-state ramp stays hot."""
                n = 0
                while prio and n < 2:
                    prio.pop(0)()
                    n += 1
                if soon and st8["pe"] < st8["act"] + 2 * LEAD:
                    fn, key = soon.pop(0)
                    fn()
                elif fills and st8["pe"] < st8["act"] + LEAD:
                    fn, key = fills.pop(0)
                    fn()
                while (not fills and not prio and st8["dummies"] < 200
                       and st8["pe"] < st8["act"]):
                    wps = psA.tile([P, 512], f32, name="wrm", tag="qp",
                                   bufs=2)
                    nc.tensor.matmul(wps, lhsT=wu_a, rhs=wu_b,
                                     start=True, stop=True,
                                     skip_group_check=True)
                    pe(512)
                    st8["dummies"] += 1

            def _force(q, cond):
                i = 0
                while i < len(q):
                    fn, key = q[i]
                    if cond(key):
                        q.pop(i)
                        fn()
                    else:
                        i += 1

            def force_ld(c):
                for q in (soon, fills):
                    _force(q, lambda k: k[0] == "ld" and k[1] <= c)

            def force_kq(c, s):
                force_ld(c)
                for q in (soon, fills):
                    _force(q, lambda k: k[0] == "kq" and k[1] <= c
                           and k[2] in (s, 0) if k[0] == "kq" else False)

            def force_v(tt, hp):
                force_ld(tt // 4)
                for q in (soon, fills):
                    _force(q, lambda k:
                           (k[0] == "v" and k[1] <= tt and k[2] == hp)
                           or (k[0] == "lw" and k[1] == hp))

            # ---- attention building blocks ----
            def s_block_emitters(hp, c):
                """Per-block closures for S^T + exp of head pair hp,
                chunk c; returns (emitters, pts) where pts is filled in
                as blocks run."""
                s_h = hp // 2
                q0, q1 = (2 * hp) % 4, (2 * hp + 1) % 4
                nfull = 4 * c
                pts = []
                ems = []

                def kq_op(kq, Q, c0, c1):
                    if Q == 3:
                        lo = 32 * s_h
                        return kq8d[lo:lo + 32, kq, :, c0:c1]
                    return kq8a[32 * Q:32 * Q + 32, kq, s_h, :, c0:c1]

                def full_block(j):
                    def em():
                        st = psA.tile([P, 2, 512], f32, name="st", tag="st",
                                      bufs=2)
                        for hi, Q in ((0, q0), (1, q1)):
                            nc.tensor.matmul(
                                st[:, hi, :],
                                lhsT=kq_op(0, Q, j * P, (j + 1) * P),
                                rhs=kq_op(1, Q, c * 512, (c + 1) * 512),
                                start=True, stop=True,
                                perf_mode=DR, skip_group_check=True)
                        pt = ptP.tile([P, 2, 512], bf16, name="pt",
                                      tag="pt")
                        nc.scalar.activation(pt, st, Exp, scale=SCALE)
                        pts.append((pt, j, 0))
                        pe(512)
                        act(2 * 1024 + 444)
                    return em

                def diag_block(dj):
                    def em():
                        j = nfull + dj
                        off = P * dj
                        w = 512 - off
                        st = psA.tile([P, 2, 512], f32, name="std",
                                      tag="st")
                        for hi, Q in ((0, q0), (1, q1)):
                            nc.tensor.matmul(
                                st[:, hi, 0:w],
                                lhsT=kq_op(0, Q, j * P, (j + 1) * P),
                                rhs=kq_op(1, Q, c * 512 + off,
                                          (c + 1) * 512),
                                start=True, stop=True,
                                perf_mode=DR, skip_group_check=True)
                        pt = ptdP.tile([P, 2, 512], bf16, name="ptd",
                                       tag="ptd")
                        nc.scalar.activation(pt[:, :, 0:w], st[:, :, 0:w],
                                             Exp, scale=SCALE)
                        nc.gpsimd.affine_select(
                            pt[:, :, 0:P], pt[:, :, 0:P],
                            pattern=[[0, 2], [1, P]],
                            compare_op=mybir.AluOpType.is_ge,
                            fill=0.0, base=0, channel_multiplier=-1)
                        pts.append((pt, j, off))
                        pe(w)
                        act(4 * w + 444)
                    return em

                for j in range(nfull):
                    ems.append(full_block(j))
                for dj in range(4):
                    ems.append(diag_block(dj))
                return ems, pts

            def av_emitters(hp, c, pts):
                """Per-head closures: 4 accumulation chains + drain."""
                ems = []

                def head(hi):
                    def em():
                        h = 2 * hp + hi
                        avp = psA.tile([P, 4, P], f32, name=f"avp{hi}",
                                       tag="avp", bufs=2)
                        rows = 0
                        for u in range(4):
                            tq = 4 * c + u
                            chain = [pj for pj in pts if pj[1] <= tq]
                            n = len(chain)
                            for idx, (pt, j, off) in enumerate(chain):
                                lo = u * P - off
                                nc.tensor.matmul(
                                    avp[:, u, 0:D + 1],
                                    lhsT=pt[:, hi, lo:lo + P],
                                    rhs=v_aug[:, j, h, :],
                                    start=(u == 0 and idx == 0),
                                    stop=(idx == n - 1),
                                    skip_group_check=True)
                                rows += D + 1
                        rc = rcP.tile([P, 4, 1], f32, name="rc", tag="rc",
                                      bufs=4)
                        nc.vector.reciprocal(rc, avp[:, :, D:D + 1])
                        nc.vector.tensor_mul(
                            att_nat[:, 4 * c:4 * c + 4, h, :],
                            avp[:, :, 0:D],
                            rc.broadcast_to([P, 4, D]))
                        pe(rows)
                    return em

                return [head(0), head(1)]

            # ---- main interleaved emission ----
            # hp order (0,2,1,3): the first chunks use heads on quarters
            # 0..2 only, so nothing waits on the kq8b fix-up DMAs while the
            # input-load train still occupies the DMA engines.
            # c order (1,2,3,0): the epilogue chunk is the small c=0 one,
            # so little work trails the final exp
            c_order = list(range(qc_n))
            chunk_seq = [(hp, c) for c in c_order for hp in (0, 2, 1, 3)]
            done_hp = set()
            pending = None   # (hp, c, av emitter list)

            attTc_map = {}

            def queue_transposes(pc, cb):
                """Transposes of head pair cb for token chunk pc -> prio,
                as soon as that pair's AV drains."""
                if pc not in attTc_map:
                    attTc_map[pc] = atP.tile([P, VB, 4, P], bf16,
                                             name="attTc", tag="attTc",
                                             bufs=2)
                a = attTc_map[pc]
                for tt in range(4 * pc, 4 * pc + 4):
                    prio.append((lambda tt=tt, cb=cb, a=a:
                                 emit_transpose(tt, cb, a)))

            def queue_proj(pc, alt=False):
                a = attTc_map.pop(pc)
                for mb in range(KT):
                    tag = "avp" if (alt and mb % 2) else "qp"
                    prio.append((lambda mb=mb, pc=pc, a=a, tag=tag:
                                 emit_proj_unit(mb, pc, a, tag)))

            for ci, (hp, c) in enumerate(chunk_seq):
                force_kq(c, hp // 2)
                s_ems, pts = s_block_emitters(hp, c)
                avq = []
                if pending is not None:
                    php, pc, avq = pending
                    force_v(4 * pc + 3, php)
                for bi, em in enumerate(s_ems):
                    em()
                    if bi == 3:
                        # pull upcoming kq units (and their loads) in
                        # under this chunk's exp cover rather than at
                        # the point of need
                        if hp == 0:
                            force_kq(c, 1)
                            force_ld(min(c + 1, qc_n - 1))
                        elif hp == 1 and c + 1 < qc_n:
                            force_kq(c + 1, 0)
                        elif hp == 3 and c + 1 < qc_n:
                            force_kq(c + 1, 1)
                    if bi >= 2 and avq:
                        avq.pop(0)()
                    emit_fills()
                while avq:
                    avq.pop(0)()
                    emit_fills()
                if pending is not None:
                    queue_transposes(pc, php)
                    done_hp.add((php, pc))
                    if all((h2, pc) in done_hp for h2 in range(4)):
                        queue_proj(pc)
                pending = (hp, c, av_emitters(hp, c, pts))

            php, pc, avq = pending
            force_v(4 * pc + 3, php)
            for em in avq:
                em()
                while prio:
                    prio.pop(0)()
            queue_transposes(pc, php)
            while prio:
                prio.pop(0)()
            queue_proj(pc, alt=True)
            while prio:
                prio.pop(0)()
            while fills:
                fn, key = fills.pop(0)
                fn()

    nc.compile()
    return nc


def _get_compiled(t=T):
    if t not in _compiled:
        _compiled[t] = _build(t)
    return _compiled[t]


def make_in_maps(x, W_qkv, W_proj):
    bf = ml_dtypes.bfloat16
    f8 = ml_dtypes.float8_e4m3
    x = np.asarray(x, dtype=np.float32)
    W_qkv = np.asarray(W_qkv, dtype=np.float32)
    W_proj = np.asarray(W_proj, dtype=np.float32)
    ident = np.eye(P, dtype=np.float32).astype(bf)
    in_maps = []
    for core in range(8):
        b, g = core // 2, core % 2
        xT = np.ascontiguousarray(x[b].T)           # [C, T]
        Wk = W_qkv[:, g * CG:(g + 1) * CG]
        Wq = W_qkv[:, C + g * CG:C + (g + 1) * CG]
        Wv = W_qkv[:, 2 * C + g * CG:2 * C + (g + 1) * CG]
        # permute k/q columns so the QKV matmul output lands directly in
        # the kq8a/kq8d layouts (see _build):
        #  per s-slot: cols [96a, 96a+96) = A-block a=2kq+e, slot-heads
        #  0..2; cols [384+64e, +64) = D-block (head 3's k then q)
        wkq8 = np.empty((C, C), dtype=np.float32)
        for kqi, W in ((0, Wk), (1, Wq)):
            Wr = (W * WS).reshape(C, 2, 4, 2, 32)      # [C, s, h', e, r]
            for s in range(2):
                for e in range(2):
                    a = 2 * kqi + e
                    lo = 384 * s + 96 * a
                    wkq8[:, lo:lo + 96] = Wr[:, s, 0:3, e, :].reshape(C, 96)
            for e in range(2):
                d = 2 * kqi + e
                lo = 768 + 64 * d
                wkq8[:, lo:lo + 32] = Wr[:, 0, 3, e, :]
                wkq8[:, lo + 32:lo + 64] = Wr[:, 1, 3, e, :]
        in_maps.append({
            "xT": xT.astype(bf),
            "x8T": xT.astype(f8),
            "wkq8": wkq8.astype(f8),
            "wv": np.ascontiguousarray(Wv).astype(bf),
            "wp": np.ascontiguousarray(
                W_proj[g * CG:(g + 1) * CG, :]).astype(bf),
            "ident": ident,
        })
    return in_maps


def _run_axon_nodonate(nc, in_maps, n_cores=8):
    """Execute via PJRT/shard_map WITHOUT output-buffer donation.

    bass2jax.run_bass_via_pjrt donates the zero output operands; under the
    axon transport that donation intermittently corrupts multi-core results.
    This kernel writes every element of its output, so donation is not
    needed for correctness -- pass non-donated zero operands instead.
    """
    import jax
    from jax.sharding import Mesh, PartitionSpec
    from jax.experimental.shard_map import shard_map
    import concourse.mybir as mybir
    from concourse.bass2jax import _bass_exec_p, install_neuronx_cc_hook

    install_neuronx_cc_hook()
    in_names, out_names, out_avals = [], [], []
    for alloc in nc.m.functions[0].allocations:
        if not isinstance(alloc, mybir.MemoryLocationSet):
            continue
        name = alloc.memorylocations[0].name
        if alloc.kind == "ExternalInput":
            in_names.append(name)
        elif alloc.kind == "ExternalOutput":
            out_names.append(name)
            out_avals.append(jax.core.ShapedArray(
                tuple(alloc.tensor_shape), mybir.dt.np(alloc.dtype)))
    n_params = len(in_names)
    all_names = in_names + out_names
    pid_name = nc.partition_id_tensor.name if nc.partition_id_tensor else None

    def _body(*args):
        return tuple(_bass_exec_p.bind(
            *args,
            out_avals=tuple(out_avals),
            in_names=tuple(all_names),
            out_names=tuple(out_names),
            lowering_input_output_aliases=(),
            sim_require_finite=True,
            sim_require_nnan=True,
            nc=nc,
        ))

    devices = jax.devices()[:n_cores]
    mesh = Mesh(np.asarray(devices), ("core",))
    fn = jax.jit(
        shard_map(_body, mesh=mesh,
                  in_specs=(PartitionSpec("core"),) * (n_params + len(out_names)),
                  out_specs=(PartitionSpec("core"),) * len(out_names),
                  check_rep=False),
        keep_unused=True)
    concat_in = [
        np.concatenate([
            np.asarray(in_maps[c].get(
                nm, np.array([[c]], dtype=np.uint32) if nm == pid_name
                else None))
            for c in range(n_cores)], 0)
        for nm in in_names
    ]
    concat_zeros = [
        np.zeros((n_cores * a.shape[0], *a.shape[1:]), a.dtype)
        for a in out_avals
    ]
    out = fn(*concat_in, *concat_zeros)
    return [
        {nm: np.asarray(out[i]).reshape(n_cores, *out_avals[i].shape)[c]
         for i, nm in enumerate(out_names)}
        for c in range(n_cores)
    ]


def kernel(x, W_qkv, W_proj, _trace=False):
    from concourse._compat import axon_active

    nc = _get_compiled()
    in_maps = make_in_maps(x, W_qkv, W_proj)
    if axon_active():
        results = _run_axon_nodonate(nc, in_maps)
    else:
        import concourse.bass_utils as bass_utils
        res = bass_utils.run_bass_kernel_spmd(
            nc, in_maps, core_ids=list(range(8)), trace=_trace)
        if _trace:
            kernel.last_results = res
        results = res.results
    y = np.zeros((B, T, C), np.float32)
    for core in range(8):
        y[core // 2] += results[core]["y"].T
    return y
